# revision 21
# baseline (speedup 1.0000x reference)
"""Trainium2 Bass kernel for nn_AttenLSTMModel (2-layer LSTM + last-query
attention + FC head).

kernel(**inputs): FULL numpy inputs (B=64) -> FULL [64, 128] output.
Batch sharded 8-way across the 8 NeuronCores (pure SPMD, one NEFF).

v1.5 design notes (per core, local batch BL=8, token index = t*BL + b):
 - Weights arrive HOST-pre-transposed (and 0.5-prescaled where needed).
 - All gate activations are Tanh-only (sigmoid(z) = 0.5*tanh(z/2)+0.5) so
   the ACT engine never reloads its LUT table. Affine factors folded:
   track C2 = 2c and h2 = 2h; pre-scale Whh^T, Wih1^T, Wq/Wk/Wv^T by 0.5.
     A    = (th_i + 1) * th_g          (= 2 i' g')
     B    = (th_f + 1) * C2_old        (= 4 f' c_old)
     C2   = 0.5*B + A
     th_C = tanh(0.5 * C2)             (= tanh(c))
     h2   = (th_o + 1) * th_C          (= 2 h)
 - xp preloaded into PSUM via identity-selector matmul; fp32r matmuls.
 - LSTM0 / xp1-production / LSTM1 interleaved in 16-step chunks so one
   layer's matmul stream fills the other layer's elementwise-tail bubbles.
 - h_t^T history spilled to DRAM (hist0/hist1), read back in chunks.
"""

import os

os.environ.setdefault("JAX_PLATFORMS", "axon,cpu")

import numpy as np

import concourse.bass as bass
import concourse.mybir as mybir
import concourse.tile as tile
from concourse.bass_utils import run_bass_kernel_spmd
from concourse.masks import make_identity

F32 = mybir.dt.float32
F32R = mybir.dt.float32r
AF = mybir.ActivationFunctionType
ALU = mybir.AluOpType

B, T_FULL, I, H, O = 64, 512, 128, 512, 128
G = 4 * H
N_CORES = 8
BL = B // N_CORES  # 8
KC = H // 128  # 4
NC_G = G // 512  # 4
HB = KC * BL  # 32
CH = 16  # interleave chunk: steps per chunk (= 128 tokens)


def split_wide_waits(nc, max_waits=1):
    """This container's walrus rejects >1 semaphore wait per CTRL-class
    instruction. Split extra waits onto preceding same-engine drains."""
    for f in nc.m.functions:
        for bb in f.blocks:
            insts = bb.instructions
            changed = False
            out = []
            for ins in insts:
                si = getattr(ins, "sync_info", None)
                if si and si.on_wait and len(si.on_wait) > max_waits:
                    waits = list(si.on_wait)
                    extra, keep = waits[:-max_waits], waits[-max_waits:]
                    for j in range(0, len(extra), max_waits):
                        nd = mybir.InstDrain(name=f"{ins.name}-wsplit{j}")
                        nd.engine = ins.engine
                        nd.sync_info = mybir.SyncInfo(
                            on_wait=extra[j : j + max_waits], on_update=[]
                        )
                        out.append(nd)
                    ins.sync_info = mybir.SyncInfo(
                        on_wait=keep, on_update=list(si.on_update)
                    )
                    changed = True
                out.append(ins)
            if changed:
                bb.instructions = out


class LstmState:
    """Per-layer recurrence state + step emitter."""

    def __init__(self, nc, li, state, hring, psum_gates, psum_small, whhT,
                 hist_dram, sel128, gates_tile):
        self.nc = nc
        self.li = li
        self.pb = 0  # both layers share PSUM banks at partition base 0
        self.state = state
        self.hring = hring
        self.pss = psum_small
        self.whhT = whhT
        self.hist = hist_dram
        self.sel128 = sel128
        self.gates = gates_tile[self.pb : self.pb + BL]
        id8t = state.tile([40, BL], F32, tag=f"id8_{li}", name=f"id8_{li}")
        make_identity(nc, id8t[self.pb : self.pb + BL, :])
        self.ident8 = id8t[self.pb : self.pb + BL, :]
        c2t = state.tile([40, H], F32, tag=f"c{li}", name=f"c{li}")
        self.c2 = c2t[self.pb : self.pb + BL]
        nc.gpsimd.memset(self.c2[:], 0.0)
        zro = state.tile([128, KC, BL], F32, tag="zro", name="zro")
        nc.gpsimd.memset(zro[:], 0.0)
        self.h_prev = hring.tile(
            [128, KC, BL], F32R, tag=f"hT{li}", name=f"hT{li}_init"
        )
        nc.vector.tensor_copy(out=self.h_prev[:], in_=zro[:])
        self.xp_tile = None

    def _sbt(self, tag):
        t = self.state.tile([40, 512], F32, tag=f"{tag}{self.li}",
                            name=f"{tag}{self.li}")
        return t[self.pb : self.pb + BL]

    def step(self, t, xp_dram, xp_pool):
        nc = self.nc
        li = self.li
        sel = t % CH
        if sel == 0:
            self.xp_tile = xp_pool.tile(
                [128, G], F32R, tag=f"xp{li}", name=f"xp{li}c"
            )
            nc.sync.dma_start(
                self.xp_tile[:], xp_dram[t * BL : t * BL + 128, :]
            )
        gates = self.gates
        tp = (0, self.pb) if self.pb else None
        for n in range(NC_G):  # i, f, g, o slices
            nc.tensor.matmul(
                gates[:, n],
                self.sel128[:, sel * BL : (sel + 1) * BL],
                self.xp_tile[:, n * 512 : (n + 1) * 512],
                start=True,
                stop=False,
                tile_position=tp,
            )
            for k in range(KC):
                nc.tensor.matmul(
                    gates[:, n],
                    self.h_prev[:, k, :],
                    self.whhT[:, k, n * 512 : (n + 1) * 512],
                    start=False,
                    stop=(k == KC - 1),
                    tile_position=tp,
                )
        th = {}
        # i, f, o: tanh(z/2); g: tanh(z)
        for n, sc in ((0, 0.5), (2, 1.0), (1, 0.5), (3, 0.5)):
            a = self._sbt(f"th{n}_")
            th[n] = a
            nc.scalar.activation(a[:], gates[:, n], AF.Tanh, scale=sc)
        A = self._sbt("A")
        nc.vector.scalar_tensor_tensor(
            out=A[:], in0=th[0][:], scalar=1.0, in1=th[2][:],
            op0=ALU.add, op1=ALU.mult,
        )
        Bt = self._sbt("B")
        nc.vector.scalar_tensor_tensor(
            out=Bt[:], in0=th[1][:], scalar=1.0, in1=self.c2[:],
            op0=ALU.add, op1=ALU.mult,
        )
        nc.vector.scalar_tensor_tensor(
            out=self.c2[:], in0=Bt[:], scalar=0.5, in1=A[:],
            op0=ALU.mult, op1=ALU.add,
        )
        thc = self._sbt("thc")
        nc.scalar.activation(thc[:], self.c2[:], AF.Tanh, scale=0.5)
        h2 = self._sbt("h")
        nc.vector.scalar_tensor_tensor(
            out=h2[:], in0=th[3][:], scalar=1.0, in1=thc[:],
            op0=ALU.add, op1=ALU.mult,
        )
        ptr = self.pss.tile([128, KC, BL], F32, tag="ptr", name=f"ptr{li}")
        for k in range(KC):
            nc.tensor.transpose(
                ptr[:, k], h2[:, k * 128 : (k + 1) * 128], self.ident8
            )
        h_cur = self.hring.tile(
            [128, KC, BL], F32R, tag=f"hT{li}", name=f"hT{li}_c"
        )
        nc.vector.tensor_copy(out=h_cur[:], in_=ptr[:])
        nc.sync.dma_start(
            self.hist[:, :, t + 1, :].rearrange("k p b -> p k b"), h_cur[:]
        )
        self.h_prev = h_cur


def build_kernel(T=T_FULL, split=True, interleave=True):
    TOK = T * BL
    NCH = T // CH
    nc = bass.Bass("TRN2")
    x_l = nc.dram_tensor("x", [BL, T, I], F32, kind="ExternalInput")
    # host-pre-transposed weights: [128, KC_in, out_dim] with
    # w[p, k, j] = W_orig[j, k*128 + p] (times 0.5 where prescaled)
    wih0T_d = nc.dram_tensor("wih0T", [128, 1, G], F32R, kind="ExternalInput")
    whh0T_d = nc.dram_tensor("whh0T", [128, KC, G], F32R, kind="ExternalInput")
    wih1T_d = nc.dram_tensor("wih1T", [128, KC, G], F32R, kind="ExternalInput")
    whh1T_d = nc.dram_tensor("whh1T", [128, KC, G], F32R, kind="ExternalInput")
    wqT_d = nc.dram_tensor("wqT", [128, KC, H], F32R, kind="ExternalInput")
    wkT_d = nc.dram_tensor("wkT", [128, KC, H], F32R, kind="ExternalInput")
    wvT_d = nc.dram_tensor("wvT", [128, KC, H], F32R, kind="ExternalInput")
    wfcT_d = nc.dram_tensor("wfcT", [128, KC, O], F32R, kind="ExternalInput")
    b0s = nc.dram_tensor("b0s", [G], F32, kind="ExternalInput")
    b1s = nc.dram_tensor("b1s", [G], F32, kind="ExternalInput")
    bq = nc.dram_tensor("bq", [H], F32, kind="ExternalInput")
    bk = nc.dram_tensor("bk", [H], F32, kind="ExternalInput")
    bv = nc.dram_tensor("bv", [H], F32, kind="ExternalInput")
    bfc = nc.dram_tensor("bfc", [O], F32, kind="ExternalInput")
    y = nc.dram_tensor("y", [BL, O], F32, kind="ExternalOutput")

    with tile.TileContext(nc) as tc:
        with (
            tc.tile_pool(name="big", bufs=1) as big,
            tc.tile_pool(name="state", bufs=1) as state,
            tc.tile_pool(name="hring", bufs=2) as hring,
            tc.tile_pool(name="rot", bufs=3) as rot,
            tc.tile_pool(name="xp", bufs=1) as xp_pool,
            tc.tile_pool(name="hrd", bufs=2) as hrd_pool,
            tc.tile_pool(name="psg", bufs=1, space="PSUM") as psum_gates,
            tc.tile_pool(name="pss", bufs=2, space="PSUM") as psum_small,
            tc.tile_pool(name="dram", bufs=1, space="DRAM") as dram,
        ):
            # ---- identities ----
            ident128 = state.tile([128, 128], F32, tag="id128")
            make_identity(nc, ident128[:])
            ident8 = state.tile([BL, BL], F32, tag="id8")
            make_identity(nc, ident8[:])
            sel128 = state.tile([128, 128], F32R, tag="sel128")
            nc.vector.tensor_copy(out=sel128[:], in_=ident128[:])
            ones = state.tile([1, 128], F32, tag="ones")
            nc.gpsimd.memset(ones[:], 1.0)
            brow = state.tile([1, G], F32, tag="brow")

            # ---- weights (direct DMA of host-transposed forms) ----
            whh0T = big.tile([128, KC, G], F32R, tag="wA")
            nc.sync.dma_start(whh0T[:], whh0T_d[:])
            whh1T = big.tile([128, KC, G], F32R, tag="wA2")
            nc.sync.dma_start(whh1T[:], whh1T_d[:])
            wih1T = big.tile([128, KC, G], F32R, tag="wB")
            nc.sync.dma_start(wih1T[:], wih1T_d[:])
            wih0T = big.tile([128, 1, G], F32R, tag="wB0")
            nc.sync.dma_start(wih0T[:], wih0T_d[:])
            bias_bc = big.tile([128, G], F32, tag="bias")
            nc.sync.dma_start(brow[:], b0s[None, :])
            for n0 in range(0, G, 512):
                ps = psum_small.tile([128, 512], F32, tag="ps_a", name="bcps")
                nc.tensor.matmul(
                    ps[:], ones[:], brow[:, n0 : n0 + 512], start=True, stop=True
                )
                nc.vector.tensor_copy(out=bias_bc[:, n0 : n0 + 512], in_=ps[:])

            # ---- phase 1: xT + xp0 ----
            xt_cols = max(TOK, 4096)
            xT_slot = big.tile([128, xt_cols], F32R, tag="xattn")
            xT = xT_slot[:, :TOK]
            xTv = xT.rearrange("p (t b) -> p t b", b=BL)
            for b in range(BL):
                for tch in range(T // 128):
                    xb = rot.tile([128, I], F32, tag="xb", name="xb")
                    nc.sync.dma_start(xb[:], x_l[b, tch * 128 : (tch + 1) * 128, :])
                    ps = psum_small.tile([128, 128], F32, tag="ps_a", name="xps")
                    nc.tensor.transpose(ps[:], xb[:], ident128)
                    nc.vector.tensor_copy(
                        out=xTv[:, tch * 128 : (tch + 1) * 128, b], in_=ps[:, :128]
                    )
            xp_dram = dram.tile([TOK, G], F32R, tag="xp_dram")
            for mt in range(TOK // 128):
                lhsT = xT[:, mt * 128 : (mt + 1) * 128]
                for n in range(NC_G):
                    ps = psum_small.tile([128, 512], F32, tag="ps_a", name="x0ps")
                    nc.tensor.matmul(
                        ps[:], lhsT, wih0T[:, 0, n * 512 : (n + 1) * 512],
                        start=True, stop=True,
                    )
                    sb = rot.tile([128, 512], F32R, tag="xpout", name="x0sb")
                    nc.vector.tensor_add(
                        out=sb[:], in0=ps[:], in1=bias_bc[:, n * 512 : (n + 1) * 512]
                    )
                    nc.sync.dma_start(
                        xp_dram[mt * 128 : (mt + 1) * 128, n * 512 : (n + 1) * 512],
                        sb[:],
                    )
            # bias1 into the same slot (xp0 fully produced above)
            bias1_bc = big.tile([128, G], F32, tag="bias")
            nc.sync.dma_start(brow[:], b1s[None, :])
            for n0 in range(0, G, 512):
                ps = psum_small.tile([128, 512], F32, tag="ps_a", name="bc1ps")
                nc.tensor.matmul(
                    ps[:], ones[:], brow[:, n0 : n0 + 512], start=True, stop=True
                )
                nc.vector.tensor_copy(out=bias1_bc[:, n0 : n0 + 512], in_=ps[:])

            # ---- interleaved L0 / xp1 / L1 ----
            hist0 = dram.tile([KC, 128, T + 2, BL], F32R, tag="hist0")
            hist1 = dram.tile([KC, 128, T + 2, BL], F32R, tag="hist1")
            xp1_dram = dram.tile([TOK, G], F32R, tag="xp_dram2")
            gates_tile = psum_gates.tile(
                [BL, NC_G, 512], F32, tag="gates", name="gatesT"
            )
            L0 = LstmState(nc, 0, state, hring, psum_gates, psum_small,
                           whh0T, hist0, sel128, gates_tile)
            L1 = LstmState(nc, 1, state, hring, psum_gates, psum_small,
                           whh1T, hist1, sel128, gates_tile)
            if not interleave:
                for t in range(T):
                    L0.step(t, xp_dram, xp_pool)
            for j in range(NCH + 1):
                if interleave and j >= 1:
                    for t in range((j - 1) * CH, j * CH):
                        L1.step(t, xp1_dram, xp_pool)
                if interleave and j < NCH:
                    for t in range(j * CH, (j + 1) * CH):
                        L0.step(t, xp_dram, xp_pool)
                    # xp1 for chunk j (tokens j*128 .. (j+1)*128)
                    mt = j
                    hch = hrd_pool.tile([128, KC, 128], F32R, tag="hrd", name="hch")
                    nc.sync.dma_start(
                        hch[:],
                        hist0[:, :, 1 + mt * CH : 1 + (mt + 1) * CH, :].rearrange(
                            "k p s b -> p k (s b)"
                        ),
                    )
                    for n in range(NC_G):
                        ps = psum_small.tile([128, 512], F32, tag="ps_a", name="x1ps")
                        for k in range(KC):
                            nc.tensor.matmul(
                                ps[:], hch[:, k, :],
                                wih1T[:, k, n * 512 : (n + 1) * 512],
                                start=(k == 0), stop=(k == KC - 1),
                            )
                        sb = rot.tile([128, 512], F32R, tag="xpout", name="x1sb")
                        nc.vector.tensor_add(
                            out=sb[:], in0=ps[:],
                            in1=bias1_bc[:, n * 512 : (n + 1) * 512],
                        )
                        nc.sync.dma_start(
                            xp1_dram[mt * 128 : (mt + 1) * 128,
                                     n * 512 : (n + 1) * 512],
                            sb[:],
                        )
            if not interleave:
                for mt in range(NCH):
                    hch = hrd_pool.tile([128, KC, 128], F32R, tag="hrd", name="hch")
                    nc.sync.dma_start(
                        hch[:],
                        hist0[:, :, 1 + mt * CH : 1 + (mt + 1) * CH, :].rearrange(
                            "k p s b -> p k (s b)"
                        ),
                    )
                    for n in range(NC_G):
                        ps = psum_small.tile([128, 512], F32, tag="ps_a", name="x1ps")
                        for k in range(KC):
                            nc.tensor.matmul(
                                ps[:], hch[:, k, :],
                                wih1T[:, k, n * 512 : (n + 1) * 512],
                                start=(k == 0), stop=(k == KC - 1),
                            )
                        sb = rot.tile([128, 512], F32R, tag="xpout", name="x1sb")
                        nc.vector.tensor_add(
                            out=sb[:], in0=ps[:],
                            in1=bias1_bc[:, n * 512 : (n + 1) * 512],
                        )
                        nc.sync.dma_start(
                            xp1_dram[mt * 128 : (mt + 1) * 128,
                                     n * 512 : (n + 1) * 512],
                            sb[:],
                        )
                for t in range(T):
                    L1.step(t, xp1_dram, xp_pool)

            # ---- attention + fc ----
            wq_sb = big.tile([128, KC, H], F32R, tag="wA")
            nc.sync.dma_start(wq_sb[:], wqT_d[:])
            wk_sb = big.tile([128, KC, H], F32R, tag="wA2")
            nc.sync.dma_start(wk_sb[:], wkT_d[:])
            wv_sb = big.tile([128, KC, H], F32R, tag="wB")
            nc.sync.dma_start(wv_sb[:], wvT_d[:])
            wfc_sb = big.tile([128, KC, O], F32R, tag="wB0")
            nc.sync.dma_start(wfc_sb[:], wfcT_d[:])
            scale = float(1.0 / np.sqrt(H))
            bq_s = state.tile([128, KC], F32, tag="bq")
            nc.sync.dma_start(bq_s[:], bq.rearrange("(c p) -> p c", p=128))
            nc.vector.tensor_scalar_mul(bq_s[:], bq_s[:], scale)
            bk_s = state.tile([128, KC], F32, tag="bk")
            nc.sync.dma_start(bk_s[:], bk.rearrange("(c p) -> p c", p=128))
            bv_bc = big.tile([128, G], F32, tag="bias")  # [:, :H] used
            nc.sync.dma_start(brow[:, :H], bv[None, :])
            for n0 in range(0, H, 512):
                ps = psum_small.tile([128, 512], F32, tag="ps_a", name="bvps")
                nc.tensor.matmul(
                    ps[:], ones[:], brow[:, n0 : n0 + 512], start=True, stop=True
                )
                nc.vector.tensor_copy(out=bv_bc[:, n0 : n0 + 512], in_=ps[:])
            bfc_s = state.tile([128, 1], F32, tag="bfc")
            nc.sync.dma_start(bfc_s[:], bfc[:, None])

            hq = hring.tile([128, KC, BL], F32R, tag="hT1", name="hq")
            nc.sync.dma_start(
                hq[:], hist1[:, :, T, :].rearrange("k p b -> p k b")
            )
            qT = state.tile([128, KC, BL], F32R, tag="qT")
            for hoc in range(KC):
                ps = psum_small.tile([128, BL], F32, tag="ptr", name="qps")
                for k in range(KC):
                    nc.tensor.matmul(
                        ps[:], wq_sb[:, k, hoc * 128 : (hoc + 1) * 128],
                        hq[:, k, :], start=(k == 0), stop=(k == KC - 1),
                    )
                nc.scalar.activation(
                    qT[:, hoc], ps[:], AF.Identity,
                    bias=bq_s[:, hoc : hoc + 1], scale=scale,
                )
            scores = big.tile([BL, TOK], F32, tag="xattn", name="scores")
            for tk in range(TOK // 512):
                hch = xp_pool.tile([128, KC, 512], F32R, tag="xp0", name="hch5")
                nc.sync.dma_start(
                    hch[:],
                    hist1[:, :, 1 + tk * 64 : 1 + (tk + 1) * 64, :].rearrange(
                        "k p s b -> p k (s b)"
                    ),
                )
                ktc = xp_pool.tile([128, KC, 512], F32R, tag="xp1", name="ktc")
                for hoc in range(KC):
                    ps = psum_small.tile([128, 512], F32, tag="ps_a", name="kps")
                    for k in range(KC):
                        nc.tensor.matmul(
                            ps[:], wk_sb[:, k, hoc * 128 : (hoc + 1) * 128],
                            hch[:, k, :], start=(k == 0), stop=(k == KC - 1),
                        )
                    nc.scalar.activation(
                        ktc[:, hoc], ps[:], AF.Identity,
                        bias=bk_s[:, hoc : hoc + 1],
                    )
                scp = psum_small.tile([BL, 512], F32, tag="ps_a", name="scp")
                for k in range(KC):
                    nc.tensor.matmul(
                        scp[:], qT[:, k, :], ktc[:, k, :],
                        start=(k == 0), stop=(k == KC - 1),
                    )
                nc.vector.tensor_copy(
                    out=scores[:, tk * 512 : (tk + 1) * 512], in_=scp[:]
                )
            # masked softmax on the b-diagonal (mask = ident8 broadcast over t)
            BIGC = 30000.0
            attn = scores
            av = attn.rearrange("p (t b) -> p t b", b=BL)
            nc.vector.scalar_tensor_tensor(
                out=av[:], in0=av[:], scalar=BIGC,
                in1=ident8[:, None, :].to_broadcast((BL, T, BL)),
                op0=ALU.add, op1=ALU.mult,
            )
            negmax = state.tile([BL, 1], F32, tag="negmax")
            nc.vector.tensor_reduce(
                out=negmax[:], in_=attn[:],
                axis=mybir.AxisListType.X, op=ALU.max, negate=True,
            )
            sumexp = state.tile([BL, 1], F32, tag="sumexp")
            nc.scalar.activation(
                attn[:], attn[:], AF.Exp, bias=negmax[:], accum_out=sumexp[:]
            )
            rec = state.tile([BL, 1], F32, tag="rec")
            nc.vector.reciprocal(out=rec[:], in_=sumexp[:])
            nc.vector.tensor_scalar_mul(attn[:], attn[:], rec[:])
            attnT = state.tile([128, TOK // 128, BL], F32R, tag="attnT")
            for tk2 in range(TOK // 128):
                ps = psum_small.tile([128, BL], F32, tag="ptr", name="aps")
                nc.tensor.transpose(
                    ps[:], attn[:, tk2 * 128 : (tk2 + 1) * 128], ident8
                )
                nc.vector.tensor_copy(out=attnT[:, tk2, :], in_=ps[:])
            ctx_tile = psum_gates.tile(
                [128, NC_G, 512], F32, tag="gates", name="ctxps"
            )
            ctx_ps = [ctx_tile[:, hoc, 0:BL] for hoc in range(KC)]
            for tk2 in range(TOK // 128):
                hch = hrd_pool.tile([128, KC, 128], F32R, tag="hrd", name="hch5b")
                nc.sync.dma_start(
                    hch[:],
                    hist1[:, :, 1 + tk2 * CH : 1 + (tk2 + 1) * CH, :].rearrange(
                        "k p s b -> p k (s b)"
                    ),
                )
                vps = psum_small.tile([128, 512], F32, tag="ps_a", name="vps")
                for k in range(KC):
                    nc.tensor.matmul(
                        vps[:], hch[:, k, :], wv_sb[:, k, :],
                        start=(k == 0), stop=(k == KC - 1),
                    )
                vc = rot.tile([128, 512], F32R, tag="xpout", name="vc")
                nc.vector.tensor_add(out=vc[:], in0=vps[:], in1=bv_bc[:, :H])
                for hoc in range(KC):
                    nc.tensor.matmul(
                        ctx_ps[hoc], vc[:, hoc * 128 : (hoc + 1) * 128],
                        attnT[:, tk2, :],
                        start=(tk2 == 0), stop=(tk2 == TOK // 128 - 1),
                    )
            ctxT = state.tile([128, KC, BL], F32R, tag="ctxT")
            for hoc in range(KC):
                nc.scalar.activation(ctxT[:, hoc], ctx_ps[hoc], AF.Copy)
            ops = psum_small.tile([128, BL], F32, tag="ptr", name="ops")
            for k in range(KC):
                nc.tensor.matmul(
                    ops[:], wfc_sb[:, k, :], ctxT[:, k, :],
                    start=(k == 0), stop=(k == KC - 1),
                )
            outT_sb = state.tile([128, BL], F32, tag="outTsb")
            nc.scalar.activation(outT_sb[:], ops[:], AF.Identity, bias=bfc_s[:])
            fps = psum_small.tile([BL, O], F32, tag="ps_a", name="fps")
            nc.tensor.transpose(fps[:], outT_sb[:], ident128)
            y_sb = state.tile([BL, O], F32, tag="ysb")
            nc.vector.tensor_copy(out=y_sb[:], in_=fps[:])
            nc.sync.dma_start(y[:], y_sb[:])

    if split:
        split_wide_waits(nc)
    return nc


_NC_CACHE = {}


def _get_nc(T=T_FULL):
    if T not in _NC_CACHE:
        _NC_CACHE[T] = build_kernel(T)
    return _NC_CACHE[T]


def _prep_t(w, scale=1.0):
    """[out_dim, in_dim] -> [128, in_dim//128, out_dim] host pre-transpose."""
    out_dim, in_dim = w.shape
    wt = (w.T * scale).astype(np.float32)  # [in, out]
    return np.ascontiguousarray(
        wt.reshape(in_dim // 128, 128, out_dim).transpose(1, 0, 2)
    )


def prepare_host_inputs(inputs):
    m = {}
    m["wih0T"] = _prep_t(np.asarray(inputs["W_ih0"])).reshape(128, 1, G)
    m["whh0T"] = _prep_t(np.asarray(inputs["W_hh0"]), 0.5)
    m["wih1T"] = _prep_t(np.asarray(inputs["W_ih1"]), 0.5)
    m["whh1T"] = _prep_t(np.asarray(inputs["W_hh1"]), 0.5)
    m["wqT"] = _prep_t(np.asarray(inputs["Wq"]), 0.5)
    m["wkT"] = _prep_t(np.asarray(inputs["Wk"]), 0.5)
    m["wvT"] = _prep_t(np.asarray(inputs["Wv"]), 0.5)
    m["wfcT"] = _prep_t(np.asarray(inputs["Wfc"]))
    m["b0s"] = (np.asarray(inputs["b_ih0"]) + np.asarray(inputs["b_hh0"])).astype(
        np.float32
    )
    m["b1s"] = (np.asarray(inputs["b_ih1"]) + np.asarray(inputs["b_hh1"])).astype(
        np.float32
    )
    for n in ["bq", "bk", "bv", "bfc"]:
        m[n] = np.ascontiguousarray(np.asarray(inputs[n], dtype=np.float32))
    return m


def build_in_maps(inputs):
    x = np.asarray(inputs["x"], dtype=np.float32)
    shared = prepare_host_inputs(inputs)
    in_maps = []
    for c in range(N_CORES):
        m = dict(shared)
        m["x"] = np.ascontiguousarray(x[c * BL : (c + 1) * BL])
        in_maps.append(m)
    return in_maps


def kernel(**inputs):
    x = np.asarray(inputs["x"])
    nc = _get_nc(x.shape[1])
    in_maps = build_in_maps(inputs)
    res = run_bass_kernel_spmd(nc, in_maps, core_ids=list(range(N_CORES)))
    return np.concatenate([res.results[c]["y"] for c in range(N_CORES)], axis=0)


# revision 22
# speedup vs baseline: 1.0385x; 1.0385x over previous
"""Trainium2 Bass kernel for nn_AttenLSTMModel (2-layer LSTM + last-query
attention + FC head).

kernel(**inputs): FULL numpy inputs (B=64) -> FULL [64, 128] output.
Batch sharded 8-way across the 8 NeuronCores (pure SPMD, one NEFF).

v1.5 design notes (per core, local batch BL=8, token index = t*BL + b):
 - Weights arrive HOST-pre-transposed (and 0.5-prescaled where needed).
 - All gate activations are Tanh-only (sigmoid(z) = 0.5*tanh(z/2)+0.5) so
   the ACT engine never reloads its LUT table. Affine factors folded:
   track C2 = 2c and h2 = 2h; pre-scale Whh^T, Wih1^T, Wq/Wk/Wv^T by 0.5.
     A    = (th_i + 1) * th_g          (= 2 i' g')
     B    = (th_f + 1) * C2_old        (= 4 f' c_old)
     C2   = 0.5*B + A
     th_C = tanh(0.5 * C2)             (= tanh(c))
     h2   = (th_o + 1) * th_C          (= 2 h)
 - xp preloaded into PSUM via identity-selector matmul; fp32r matmuls.
 - LSTM0 / xp1-production / LSTM1 interleaved in 16-step chunks so one
   layer's matmul stream fills the other layer's elementwise-tail bubbles.
 - h_t^T history spilled to DRAM (hist0/hist1), read back in chunks.
"""

import os

os.environ.setdefault("JAX_PLATFORMS", "axon,cpu")

import numpy as np

import concourse.bass as bass
import concourse.mybir as mybir
import concourse.tile as tile
from concourse.bass_utils import run_bass_kernel_spmd
from concourse.masks import make_identity

F32 = mybir.dt.float32
F32R = mybir.dt.float32r
AF = mybir.ActivationFunctionType
ALU = mybir.AluOpType

B, T_FULL, I, H, O = 64, 512, 128, 512, 128
G = 4 * H
N_CORES = 8
BL = B // N_CORES  # 8
KC = H // 128  # 4
NC_G = G // 512  # 4
HB = KC * BL  # 32
CH = 16  # interleave chunk: steps per chunk (= 128 tokens)


def split_wide_waits(nc, max_waits=1):
    """This container's walrus rejects >1 semaphore wait per CTRL-class
    instruction. Split extra waits onto preceding same-engine drains."""
    for f in nc.m.functions:
        for bb in f.blocks:
            insts = bb.instructions
            changed = False
            out = []
            for ins in insts:
                si = getattr(ins, "sync_info", None)
                if si and si.on_wait and len(si.on_wait) > max_waits:
                    waits = list(si.on_wait)
                    extra, keep = waits[:-max_waits], waits[-max_waits:]
                    for j in range(0, len(extra), max_waits):
                        nd = mybir.InstDrain(name=f"{ins.name}-wsplit{j}")
                        nd.engine = ins.engine
                        nd.sync_info = mybir.SyncInfo(
                            on_wait=extra[j : j + max_waits], on_update=[]
                        )
                        out.append(nd)
                    ins.sync_info = mybir.SyncInfo(
                        on_wait=keep, on_update=list(si.on_update)
                    )
                    changed = True
                out.append(ins)
            if changed:
                bb.instructions = out


class LstmState:
    """Per-layer recurrence state + step emitter."""

    def __init__(self, nc, li, state, hring, psum_gates, psum_small, whhT,
                 hist_dram, sel128, gates_tile):
        self.nc = nc
        self.li = li
        self.pb = 0  # both layers share PSUM banks at partition base 0
        self.state = state
        self.hring = hring
        self.pss = psum_small
        self.whhT = whhT
        self.hist = hist_dram
        self.sel128 = sel128
        self.gates = gates_tile[self.pb : self.pb + BL]
        id8t = state.tile([40, BL], F32, tag=f"id8_{li}", name=f"id8_{li}")
        make_identity(nc, id8t[self.pb : self.pb + BL, :])
        self.ident8 = id8t[self.pb : self.pb + BL, :]
        c2t = state.tile([40, H], F32, tag=f"c{li}", name=f"c{li}")
        self.c2 = c2t[self.pb : self.pb + BL]
        nc.gpsimd.memset(self.c2[:], 0.0)
        zro = state.tile([128, KC, BL], F32, tag="zro", name="zro")
        nc.gpsimd.memset(zro[:], 0.0)
        self.h_prev = hring.tile(
            [128, KC, BL], F32R, tag=f"hT{li}", name=f"hT{li}_init"
        )
        nc.vector.tensor_copy(out=self.h_prev[:], in_=zro[:])
        self.xp_tile = None

    def _sbt(self, tag):
        t = self.state.tile([40, 512], F32, tag=f"{tag}{self.li}",
                            name=f"{tag}{self.li}")
        return t[self.pb : self.pb + BL]

    def step(self, t, xp_dram, xp_pool):
        nc = self.nc
        li = self.li
        sel = t % CH
        if sel == 0:
            self.xp_tile = xp_pool.tile(
                [128, G], F32R, tag=f"xp{li}", name=f"xp{li}c"
            )
            nc.sync.dma_start(
                self.xp_tile[:], xp_dram[t * BL : t * BL + 128, :]
            )
        gates = self.gates
        tp = (0, self.pb) if self.pb else None
        for n in range(NC_G):  # i, f, g, o slices
            nc.tensor.matmul(
                gates[:, n],
                self.sel128[:, sel * BL : (sel + 1) * BL],
                self.xp_tile[:, n * 512 : (n + 1) * 512],
                start=True,
                stop=False,
                tile_position=tp,
            )
            for k in range(KC):
                nc.tensor.matmul(
                    gates[:, n],
                    self.h_prev[:, k, :],
                    self.whhT[:, k, n * 512 : (n + 1) * 512],
                    start=False,
                    stop=(k == KC - 1),
                    tile_position=tp,
                )
        th = {}
        # i, f, o: tanh(z/2); g: tanh(z)
        for n, sc in ((0, 0.5), (2, 1.0), (1, 0.5), (3, 0.5)):
            a = self._sbt(f"th{n}_")
            th[n] = a
            nc.scalar.activation(a[:], gates[:, n], AF.Tanh, scale=sc)
        A = self._sbt("A")
        nc.vector.scalar_tensor_tensor(
            out=A[:], in0=th[0][:], scalar=1.0, in1=th[2][:],
            op0=ALU.add, op1=ALU.mult,
        )
        Bt = self._sbt("B")
        nc.vector.scalar_tensor_tensor(
            out=Bt[:], in0=th[1][:], scalar=1.0, in1=self.c2[:],
            op0=ALU.add, op1=ALU.mult,
        )
        nc.vector.scalar_tensor_tensor(
            out=self.c2[:], in0=Bt[:], scalar=0.5, in1=A[:],
            op0=ALU.mult, op1=ALU.add,
        )
        thc = self._sbt("thc")
        nc.scalar.activation(thc[:], self.c2[:], AF.Tanh, scale=0.5)
        h2 = self._sbt("h")
        nc.vector.scalar_tensor_tensor(
            out=h2[:], in0=th[3][:], scalar=1.0, in1=thc[:],
            op0=ALU.add, op1=ALU.mult,
        )
        ptr = self.pss.tile([128, KC, BL], F32, tag="ptr", name=f"ptr{li}")
        for k in range(KC):
            nc.tensor.transpose(
                ptr[:, k], h2[:, k * 128 : (k + 1) * 128], self.ident8
            )
        h_cur = self.hring.tile(
            [128, KC, BL], F32R, tag=f"hT{li}", name=f"hT{li}_c"
        )
        nc.vector.tensor_copy(out=h_cur[:], in_=ptr[:])
        nc.sync.dma_start(
            self.hist[:, :, t + 1, :].rearrange("k p b -> p k b"), h_cur[:]
        )
        self.h_prev = h_cur


def build_kernel(T=T_FULL, split=True, interleave=False):
    TOK = T * BL
    NCH = T // CH
    nc = bass.Bass("TRN2")
    x_l = nc.dram_tensor("x", [BL, T, I], F32, kind="ExternalInput")
    # host-pre-transposed weights: [128, KC_in, out_dim] with
    # w[p, k, j] = W_orig[j, k*128 + p] (times 0.5 where prescaled)
    wih0T_d = nc.dram_tensor("wih0T", [128, 1, G], F32R, kind="ExternalInput")
    whh0T_d = nc.dram_tensor("whh0T", [128, KC, G], F32R, kind="ExternalInput")
    wih1T_d = nc.dram_tensor("wih1T", [128, KC, G], F32R, kind="ExternalInput")
    whh1T_d = nc.dram_tensor("whh1T", [128, KC, G], F32R, kind="ExternalInput")
    wqT_d = nc.dram_tensor("wqT", [128, KC, H], F32R, kind="ExternalInput")
    wkT_d = nc.dram_tensor("wkT", [128, KC, H], F32R, kind="ExternalInput")
    wvT_d = nc.dram_tensor("wvT", [128, KC, H], F32R, kind="ExternalInput")
    wfcT_d = nc.dram_tensor("wfcT", [128, KC, O], F32R, kind="ExternalInput")
    b0s = nc.dram_tensor("b0s", [G], F32, kind="ExternalInput")
    b1s = nc.dram_tensor("b1s", [G], F32, kind="ExternalInput")
    bq = nc.dram_tensor("bq", [H], F32, kind="ExternalInput")
    bk = nc.dram_tensor("bk", [H], F32, kind="ExternalInput")
    bv = nc.dram_tensor("bv", [H], F32, kind="ExternalInput")
    bfc = nc.dram_tensor("bfc", [O], F32, kind="ExternalInput")
    y = nc.dram_tensor("y", [BL, O], F32, kind="ExternalOutput")

    with tile.TileContext(nc) as tc:
        with (
            tc.tile_pool(name="big", bufs=1) as big,
            tc.tile_pool(name="state", bufs=1) as state,
            tc.tile_pool(name="hring", bufs=2) as hring,
            tc.tile_pool(name="rot", bufs=3) as rot,
            tc.tile_pool(name="xp", bufs=1) as xp_pool,
            tc.tile_pool(name="hrd", bufs=2) as hrd_pool,
            tc.tile_pool(name="psg", bufs=1, space="PSUM") as psum_gates,
            tc.tile_pool(name="pss", bufs=2, space="PSUM") as psum_small,
            tc.tile_pool(name="dram", bufs=1, space="DRAM") as dram,
        ):
            # ---- identities ----
            ident128 = state.tile([128, 128], F32, tag="id128")
            make_identity(nc, ident128[:])
            ident8 = state.tile([BL, BL], F32, tag="id8")
            make_identity(nc, ident8[:])
            sel128 = state.tile([128, 128], F32R, tag="sel128")
            nc.vector.tensor_copy(out=sel128[:], in_=ident128[:])
            ones = state.tile([1, 128], F32, tag="ones")
            nc.gpsimd.memset(ones[:], 1.0)
            brow = state.tile([1, G], F32, tag="brow")

            # ---- weights (direct DMA of host-transposed forms) ----
            whh0T = big.tile([128, KC, G], F32R, tag="wA")
            nc.sync.dma_start(whh0T[:], whh0T_d[:])
            whh1T = big.tile([128, KC, G], F32R, tag="wA2")
            nc.sync.dma_start(whh1T[:], whh1T_d[:])
            wih1T = big.tile([128, KC, G], F32R, tag="wB")
            nc.sync.dma_start(wih1T[:], wih1T_d[:])
            wih0T = big.tile([128, 1, G], F32R, tag="wB0")
            nc.sync.dma_start(wih0T[:], wih0T_d[:])
            bias_bc = big.tile([128, G], F32, tag="bias")
            nc.sync.dma_start(brow[:], b0s[None, :])
            for n0 in range(0, G, 512):
                ps = psum_small.tile([128, 512], F32, tag="ps_a", name="bcps")
                nc.tensor.matmul(
                    ps[:], ones[:], brow[:, n0 : n0 + 512], start=True, stop=True
                )
                nc.vector.tensor_copy(out=bias_bc[:, n0 : n0 + 512], in_=ps[:])

            # ---- phase 1: xT + xp0 ----
            xt_cols = max(TOK, 4096)
            xT_slot = big.tile([128, xt_cols], F32R, tag="xattn")
            xT = xT_slot[:, :TOK]
            xTv = xT.rearrange("p (t b) -> p t b", b=BL)
            for b in range(BL):
                for tch in range(T // 128):
                    xb = rot.tile([128, I], F32, tag="xb", name="xb")
                    nc.sync.dma_start(xb[:], x_l[b, tch * 128 : (tch + 1) * 128, :])
                    ps = psum_small.tile([128, 128], F32, tag="ps_a", name="xps")
                    nc.tensor.transpose(ps[:], xb[:], ident128)
                    nc.vector.tensor_copy(
                        out=xTv[:, tch * 128 : (tch + 1) * 128, b], in_=ps[:, :128]
                    )
            xp_dram = dram.tile([TOK, G], F32R, tag="xp_dram")
            for mt in range(TOK // 128):
                lhsT = xT[:, mt * 128 : (mt + 1) * 128]
                for n in range(NC_G):
                    ps = psum_small.tile([128, 512], F32, tag="ps_a", name="x0ps")
                    nc.tensor.matmul(
                        ps[:], lhsT, wih0T[:, 0, n * 512 : (n + 1) * 512],
                        start=True, stop=True,
                    )
                    sb = rot.tile([128, 512], F32R, tag="xpout", name="x0sb")
                    nc.vector.tensor_add(
                        out=sb[:], in0=ps[:], in1=bias_bc[:, n * 512 : (n + 1) * 512]
                    )
                    nc.sync.dma_start(
                        xp_dram[mt * 128 : (mt + 1) * 128, n * 512 : (n + 1) * 512],
                        sb[:],
                    )
            # bias1 into the same slot (xp0 fully produced above)
            bias1_bc = big.tile([128, G], F32, tag="bias")
            nc.sync.dma_start(brow[:], b1s[None, :])
            for n0 in range(0, G, 512):
                ps = psum_small.tile([128, 512], F32, tag="ps_a", name="bc1ps")
                nc.tensor.matmul(
                    ps[:], ones[:], brow[:, n0 : n0 + 512], start=True, stop=True
                )
                nc.vector.tensor_copy(out=bias1_bc[:, n0 : n0 + 512], in_=ps[:])

            # ---- interleaved L0 / xp1 / L1 ----
            hist0 = dram.tile([KC, 128, T + 2, BL], F32R, tag="hist0")
            hist1 = dram.tile([KC, 128, T + 2, BL], F32R, tag="hist1")
            xp1_dram = dram.tile([TOK, G], F32R, tag="xp_dram2")
            gates_tile = psum_gates.tile(
                [BL, NC_G, 512], F32, tag="gates", name="gatesT"
            )
            L0 = LstmState(nc, 0, state, hring, psum_gates, psum_small,
                           whh0T, hist0, sel128, gates_tile)
            L1 = LstmState(nc, 1, state, hring, psum_gates, psum_small,
                           whh1T, hist1, sel128, gates_tile)
            if not interleave:
                for t in range(T):
                    L0.step(t, xp_dram, xp_pool)
            for j in range(NCH + 1):
                if interleave and j >= 1:
                    for t in range((j - 1) * CH, j * CH):
                        L1.step(t, xp1_dram, xp_pool)
                if interleave and j < NCH:
                    for t in range(j * CH, (j + 1) * CH):
                        L0.step(t, xp_dram, xp_pool)
                    # xp1 for chunk j (tokens j*128 .. (j+1)*128)
                    mt = j
                    hch = hrd_pool.tile([128, KC, 128], F32R, tag="hrd", name="hch")
                    nc.sync.dma_start(
                        hch[:],
                        hist0[:, :, 1 + mt * CH : 1 + (mt + 1) * CH, :].rearrange(
                            "k p s b -> p k (s b)"
                        ),
                    )
                    for n in range(NC_G):
                        ps = psum_small.tile([128, 512], F32, tag="ps_a", name="x1ps")
                        for k in range(KC):
                            nc.tensor.matmul(
                                ps[:], hch[:, k, :],
                                wih1T[:, k, n * 512 : (n + 1) * 512],
                                start=(k == 0), stop=(k == KC - 1),
                            )
                        sb = rot.tile([128, 512], F32R, tag="xpout", name="x1sb")
                        nc.vector.tensor_add(
                            out=sb[:], in0=ps[:],
                            in1=bias1_bc[:, n * 512 : (n + 1) * 512],
                        )
                        nc.sync.dma_start(
                            xp1_dram[mt * 128 : (mt + 1) * 128,
                                     n * 512 : (n + 1) * 512],
                            sb[:],
                        )
            if not interleave:
                for mt in range(NCH):
                    hch = hrd_pool.tile([128, KC, 128], F32R, tag="hrd", name="hch")
                    nc.sync.dma_start(
                        hch[:],
                        hist0[:, :, 1 + mt * CH : 1 + (mt + 1) * CH, :].rearrange(
                            "k p s b -> p k (s b)"
                        ),
                    )
                    for n in range(NC_G):
                        ps = psum_small.tile([128, 512], F32, tag="ps_a", name="x1ps")
                        for k in range(KC):
                            nc.tensor.matmul(
                                ps[:], hch[:, k, :],
                                wih1T[:, k, n * 512 : (n + 1) * 512],
                                start=(k == 0), stop=(k == KC - 1),
                            )
                        sb = rot.tile([128, 512], F32R, tag="xpout", name="x1sb")
                        nc.vector.tensor_add(
                            out=sb[:], in0=ps[:],
                            in1=bias1_bc[:, n * 512 : (n + 1) * 512],
                        )
                        nc.sync.dma_start(
                            xp1_dram[mt * 128 : (mt + 1) * 128,
                                     n * 512 : (n + 1) * 512],
                            sb[:],
                        )
                for t in range(T):
                    L1.step(t, xp1_dram, xp_pool)

            # ---- attention + fc ----
            wq_sb = big.tile([128, KC, H], F32R, tag="wA")
            nc.sync.dma_start(wq_sb[:], wqT_d[:])
            wk_sb = big.tile([128, KC, H], F32R, tag="wA2")
            nc.sync.dma_start(wk_sb[:], wkT_d[:])
            wv_sb = big.tile([128, KC, H], F32R, tag="wB")
            nc.sync.dma_start(wv_sb[:], wvT_d[:])
            wfc_sb = big.tile([128, KC, O], F32R, tag="wB0")
            nc.sync.dma_start(wfc_sb[:], wfcT_d[:])
            scale = float(1.0 / np.sqrt(H))
            bq_s = state.tile([128, KC], F32, tag="bq")
            nc.sync.dma_start(bq_s[:], bq.rearrange("(c p) -> p c", p=128))
            nc.vector.tensor_scalar_mul(bq_s[:], bq_s[:], scale)
            bk_s = state.tile([128, KC], F32, tag="bk")
            nc.sync.dma_start(bk_s[:], bk.rearrange("(c p) -> p c", p=128))
            bv_bc = big.tile([128, G], F32, tag="bias")  # [:, :H] used
            nc.sync.dma_start(brow[:, :H], bv[None, :])
            for n0 in range(0, H, 512):
                ps = psum_small.tile([128, 512], F32, tag="ps_a", name="bvps")
                nc.tensor.matmul(
                    ps[:], ones[:], brow[:, n0 : n0 + 512], start=True, stop=True
                )
                nc.vector.tensor_copy(out=bv_bc[:, n0 : n0 + 512], in_=ps[:])
            bfc_s = state.tile([128, 1], F32, tag="bfc")
            nc.sync.dma_start(bfc_s[:], bfc[:, None])

            hq = hring.tile([128, KC, BL], F32R, tag="hT1", name="hq")
            nc.sync.dma_start(
                hq[:], hist1[:, :, T, :].rearrange("k p b -> p k b")
            )
            qT = state.tile([128, KC, BL], F32R, tag="qT")
            for hoc in range(KC):
                ps = psum_small.tile([128, BL], F32, tag="ptr", name="qps")
                for k in range(KC):
                    nc.tensor.matmul(
                        ps[:], wq_sb[:, k, hoc * 128 : (hoc + 1) * 128],
                        hq[:, k, :], start=(k == 0), stop=(k == KC - 1),
                    )
                nc.scalar.activation(
                    qT[:, hoc], ps[:], AF.Identity,
                    bias=bq_s[:, hoc : hoc + 1], scale=scale,
                )
            scores = big.tile([BL, TOK], F32, tag="xattn", name="scores")
            for tk in range(TOK // 512):
                hch = xp_pool.tile([128, KC, 512], F32R, tag="xp0", name="hch5")
                nc.sync.dma_start(
                    hch[:],
                    hist1[:, :, 1 + tk * 64 : 1 + (tk + 1) * 64, :].rearrange(
                        "k p s b -> p k (s b)"
                    ),
                )
                ktc = xp_pool.tile([128, KC, 512], F32R, tag="xp1", name="ktc")
                for hoc in range(KC):
                    ps = psum_small.tile([128, 512], F32, tag="ps_a", name="kps")
                    for k in range(KC):
                        nc.tensor.matmul(
                            ps[:], wk_sb[:, k, hoc * 128 : (hoc + 1) * 128],
                            hch[:, k, :], start=(k == 0), stop=(k == KC - 1),
                        )
                    nc.scalar.activation(
                        ktc[:, hoc], ps[:], AF.Identity,
                        bias=bk_s[:, hoc : hoc + 1],
                    )
                scp = psum_small.tile([BL, 512], F32, tag="ps_a", name="scp")
                for k in range(KC):
                    nc.tensor.matmul(
                        scp[:], qT[:, k, :], ktc[:, k, :],
                        start=(k == 0), stop=(k == KC - 1),
                    )
                nc.vector.tensor_copy(
                    out=scores[:, tk * 512 : (tk + 1) * 512], in_=scp[:]
                )
            # masked softmax on the b-diagonal (mask = ident8 broadcast over t)
            BIGC = 30000.0
            attn = scores
            av = attn.rearrange("p (t b) -> p t b", b=BL)
            nc.vector.scalar_tensor_tensor(
                out=av[:], in0=av[:], scalar=BIGC,
                in1=ident8[:, None, :].to_broadcast((BL, T, BL)),
                op0=ALU.add, op1=ALU.mult,
            )
            negmax = state.tile([BL, 1], F32, tag="negmax")
            nc.vector.tensor_reduce(
                out=negmax[:], in_=attn[:],
                axis=mybir.AxisListType.X, op=ALU.max, negate=True,
            )
            sumexp = state.tile([BL, 1], F32, tag="sumexp")
            nc.scalar.activation(
                attn[:], attn[:], AF.Exp, bias=negmax[:], accum_out=sumexp[:]
            )
            rec = state.tile([BL, 1], F32, tag="rec")
            nc.vector.reciprocal(out=rec[:], in_=sumexp[:])
            nc.vector.tensor_scalar_mul(attn[:], attn[:], rec[:])
            attnT = state.tile([128, TOK // 128, BL], F32R, tag="attnT")
            for tk2 in range(TOK // 128):
                ps = psum_small.tile([128, BL], F32, tag="ptr", name="aps")
                nc.tensor.transpose(
                    ps[:], attn[:, tk2 * 128 : (tk2 + 1) * 128], ident8
                )
                nc.vector.tensor_copy(out=attnT[:, tk2, :], in_=ps[:])
            ctx_tile = psum_gates.tile(
                [128, NC_G, 512], F32, tag="gates", name="ctxps"
            )
            ctx_ps = [ctx_tile[:, hoc, 0:BL] for hoc in range(KC)]
            for tk2 in range(TOK // 128):
                hch = hrd_pool.tile([128, KC, 128], F32R, tag="hrd", name="hch5b")
                nc.sync.dma_start(
                    hch[:],
                    hist1[:, :, 1 + tk2 * CH : 1 + (tk2 + 1) * CH, :].rearrange(
                        "k p s b -> p k (s b)"
                    ),
                )
                vps = psum_small.tile([128, 512], F32, tag="ps_a", name="vps")
                for k in range(KC):
                    nc.tensor.matmul(
                        vps[:], hch[:, k, :], wv_sb[:, k, :],
                        start=(k == 0), stop=(k == KC - 1),
                    )
                vc = rot.tile([128, 512], F32R, tag="xpout", name="vc")
                nc.vector.tensor_add(out=vc[:], in0=vps[:], in1=bv_bc[:, :H])
                for hoc in range(KC):
                    nc.tensor.matmul(
                        ctx_ps[hoc], vc[:, hoc * 128 : (hoc + 1) * 128],
                        attnT[:, tk2, :],
                        start=(tk2 == 0), stop=(tk2 == TOK // 128 - 1),
                    )
            ctxT = state.tile([128, KC, BL], F32R, tag="ctxT")
            for hoc in range(KC):
                nc.scalar.activation(ctxT[:, hoc], ctx_ps[hoc], AF.Copy)
            ops = psum_small.tile([128, BL], F32, tag="ptr", name="ops")
            for k in range(KC):
                nc.tensor.matmul(
                    ops[:], wfc_sb[:, k, :], ctxT[:, k, :],
                    start=(k == 0), stop=(k == KC - 1),
                )
            outT_sb = state.tile([128, BL], F32, tag="outTsb")
            nc.scalar.activation(outT_sb[:], ops[:], AF.Identity, bias=bfc_s[:])
            fps = psum_small.tile([BL, O], F32, tag="ps_a", name="fps")
            nc.tensor.transpose(fps[:], outT_sb[:], ident128)
            y_sb = state.tile([BL, O], F32, tag="ysb")
            nc.vector.tensor_copy(out=y_sb[:], in_=fps[:])
            nc.sync.dma_start(y[:], y_sb[:])

    if split:
        split_wide_waits(nc)
    return nc


_NC_CACHE = {}


def _get_nc(T=T_FULL):
    if T not in _NC_CACHE:
        _NC_CACHE[T] = build_kernel(T)
    return _NC_CACHE[T]


def _prep_t(w, scale=1.0):
    """[out_dim, in_dim] -> [128, in_dim//128, out_dim] host pre-transpose."""
    out_dim, in_dim = w.shape
    wt = (w.T * scale).astype(np.float32)  # [in, out]
    return np.ascontiguousarray(
        wt.reshape(in_dim // 128, 128, out_dim).transpose(1, 0, 2)
    )


def prepare_host_inputs(inputs):
    m = {}
    m["wih0T"] = _prep_t(np.asarray(inputs["W_ih0"])).reshape(128, 1, G)
    m["whh0T"] = _prep_t(np.asarray(inputs["W_hh0"]), 0.5)
    m["wih1T"] = _prep_t(np.asarray(inputs["W_ih1"]), 0.5)
    m["whh1T"] = _prep_t(np.asarray(inputs["W_hh1"]), 0.5)
    m["wqT"] = _prep_t(np.asarray(inputs["Wq"]), 0.5)
    m["wkT"] = _prep_t(np.asarray(inputs["Wk"]), 0.5)
    m["wvT"] = _prep_t(np.asarray(inputs["Wv"]), 0.5)
    m["wfcT"] = _prep_t(np.asarray(inputs["Wfc"]))
    m["b0s"] = (np.asarray(inputs["b_ih0"]) + np.asarray(inputs["b_hh0"])).astype(
        np.float32
    )
    m["b1s"] = (np.asarray(inputs["b_ih1"]) + np.asarray(inputs["b_hh1"])).astype(
        np.float32
    )
    for n in ["bq", "bk", "bv", "bfc"]:
        m[n] = np.ascontiguousarray(np.asarray(inputs[n], dtype=np.float32))
    return m


def build_in_maps(inputs):
    x = np.asarray(inputs["x"], dtype=np.float32)
    shared = prepare_host_inputs(inputs)
    in_maps = []
    for c in range(N_CORES):
        m = dict(shared)
        m["x"] = np.ascontiguousarray(x[c * BL : (c + 1) * BL])
        in_maps.append(m)
    return in_maps


def kernel(**inputs):
    x = np.asarray(inputs["x"])
    nc = _get_nc(x.shape[1])
    in_maps = build_in_maps(inputs)
    res = run_bass_kernel_spmd(nc, in_maps, core_ids=list(range(N_CORES)))
    return np.concatenate([res.results[c]["y"] for c in range(N_CORES)], axis=0)


# revision 23
# speedup vs baseline: 1.6365x; 1.5758x over previous
"""Trainium2 Bass kernel for nn_AttenLSTMModel (2-layer LSTM + last-query
attention + FC head).

kernel(**inputs): FULL numpy inputs (B=64) -> FULL [64, 128] output.
Batch sharded 8-way across the 8 NeuronCores (pure SPMD, one NEFF).

v1.5 design notes (per core, local batch BL=8, token index = t*BL + b):
 - Weights arrive HOST-pre-transposed (and 0.5-prescaled where needed).
 - All gate activations are Tanh-only (sigmoid(z) = 0.5*tanh(z/2)+0.5) so
   the ACT engine never reloads its LUT table. Affine factors folded:
   track C2 = 2c and h2 = 2h; pre-scale Whh^T, Wih1^T, Wq/Wk/Wv^T by 0.5.
     A    = (th_i + 1) * th_g          (= 2 i' g')
     B    = (th_f + 1) * C2_old        (= 4 f' c_old)
     C2   = 0.5*B + A
     th_C = tanh(0.5 * C2)             (= tanh(c))
     h2   = (th_o + 1) * th_C          (= 2 h)
 - xp preloaded into PSUM via identity-selector matmul; fp32r matmuls.
 - LSTM0 / xp1-production / LSTM1 interleaved in 16-step chunks so one
   layer's matmul stream fills the other layer's elementwise-tail bubbles.
 - h_t^T history spilled to DRAM (hist0/hist1), read back in chunks.
"""

import os

os.environ.setdefault("JAX_PLATFORMS", "axon,cpu")

import numpy as np

import concourse.bass as bass
import concourse.mybir as mybir
import concourse.tile as tile
from concourse.bass_utils import run_bass_kernel_spmd
from concourse.masks import make_identity

F32 = mybir.dt.float32
F32R = mybir.dt.float32r
AF = mybir.ActivationFunctionType
ALU = mybir.AluOpType

B, T_FULL, I, H, O = 64, 512, 128, 512, 128
G = 4 * H
N_CORES = 8
BL = B // N_CORES  # 8
KC = H // 128  # 4
NC_G = G // 512  # 4
HB = KC * BL  # 32
CH = 16  # interleave chunk: steps per chunk (= 128 tokens)


def split_wide_waits(nc, max_waits=1):
    """This container's walrus rejects >1 semaphore wait per CTRL-class
    instruction. Split extra waits onto preceding same-engine drains."""
    for f in nc.m.functions:
        for bb in f.blocks:
            insts = bb.instructions
            changed = False
            out = []
            for ins in insts:
                si = getattr(ins, "sync_info", None)
                if si and si.on_wait and len(si.on_wait) > max_waits:
                    waits = list(si.on_wait)
                    extra, keep = waits[:-max_waits], waits[-max_waits:]
                    for j in range(0, len(extra), max_waits):
                        nd = mybir.InstDrain(name=f"{ins.name}-wsplit{j}")
                        nd.engine = ins.engine
                        nd.sync_info = mybir.SyncInfo(
                            on_wait=extra[j : j + max_waits], on_update=[]
                        )
                        out.append(nd)
                    ins.sync_info = mybir.SyncInfo(
                        on_wait=keep, on_update=list(si.on_update)
                    )
                    changed = True
                out.append(ins)
            if changed:
                bb.instructions = out


class LstmState:
    """Per-layer recurrence state + step emitter."""

    def __init__(self, nc, li, state, hring, psum_gates, psum_small, whhT,
                 hist_dram, sel128, gates_tile):
        self.nc = nc
        self.li = li
        self.pb = 0
        self.state = state
        self.hring = hring
        self.psg = psum_gates
        self.pss = psum_small
        self.whhT = whhT
        self.hist = hist_dram
        self.sel128 = sel128
        id8t = state.tile([40, BL], F32, tag=f"id8_{li}", name=f"id8_{li}")
        make_identity(nc, id8t[self.pb : self.pb + BL, :])
        self.ident8 = id8t[self.pb : self.pb + BL, :]
        c2t = state.tile([40, H], F32, tag=f"c{li}", name=f"c{li}")
        self.c2 = c2t[self.pb : self.pb + BL]
        nc.gpsimd.memset(self.c2[:], 0.0)
        zro = state.tile([128, KC, BL], F32, tag="zro", name="zro")
        nc.gpsimd.memset(zro[:], 0.0)
        self.h_prev = hring.tile(
            [128, KC, BL], F32R, tag=f"hT{li}", name=f"hT{li}_init"
        )
        nc.vector.tensor_copy(out=self.h_prev[:], in_=zro[:])
        self.xp_tile = None

    def _sbt(self, tag):
        t = self.state.tile([40, 512], F32, tag=f"{tag}{self.li}",
                            name=f"{tag}{self.li}")
        return t[self.pb : self.pb + BL]

    def step(self, t, xp_dram, xp_pool):
        nc = self.nc
        li = self.li
        sel = t % CH
        if sel == 0:
            self.xp_tile = xp_pool.tile(
                [128, G], F32R, tag="xp", name=f"xp{li}c"
            )
            nc.sync.dma_start(
                self.xp_tile[:], xp_dram[t * BL : t * BL + 128, :]
            )
        gates = {}
        for n in (0, 2, 1, 3):  # i, f, g, o slices; issue order i,g,f,o
            gp = self.psg.tile([BL, 512], F32, tag=f"gate{n}", name=f"g{n}_{self.li}")
            gates[n] = gp
            nc.tensor.matmul(
                gp[:],
                self.sel128[:, sel * BL : (sel + 1) * BL],
                self.xp_tile[:, n * 512 : (n + 1) * 512],
                start=True,
                stop=False,
            )
            for k in range(KC):
                nc.tensor.matmul(
                    gp[:],
                    self.h_prev[:, k, :],
                    self.whhT[:, k, n * 512 : (n + 1) * 512],
                    start=False,
                    stop=(k == KC - 1),
                )
        th = {}
        # i, f, o: tanh(z/2); g: tanh(z)
        for n, sc in ((0, 0.5), (2, 1.0), (1, 0.5), (3, 0.5)):
            a = self._sbt(f"th{n}_")
            th[n] = a
            nc.scalar.activation(a[:], gates[n][:], AF.Tanh, scale=sc)
        A = self._sbt("A")
        nc.vector.scalar_tensor_tensor(
            out=A[:], in0=th[0][:], scalar=1.0, in1=th[2][:],
            op0=ALU.add, op1=ALU.mult,
        )
        Bt = self._sbt("B")
        nc.vector.scalar_tensor_tensor(
            out=Bt[:], in0=th[1][:], scalar=1.0, in1=self.c2[:],
            op0=ALU.add, op1=ALU.mult,
        )
        nc.vector.scalar_tensor_tensor(
            out=self.c2[:], in0=Bt[:], scalar=0.5, in1=A[:],
            op0=ALU.mult, op1=ALU.add,
        )
        thc = self._sbt("thc")
        nc.scalar.activation(thc[:], self.c2[:], AF.Tanh, scale=0.5)
        h2 = self._sbt("h")
        nc.vector.scalar_tensor_tensor(
            out=h2[:], in0=th[3][:], scalar=1.0, in1=thc[:],
            op0=ALU.add, op1=ALU.mult,
        )
        ptr = self.pss.tile([128, KC, BL], F32, tag="ptr", name=f"ptr{li}")
        for k in range(KC):
            nc.tensor.transpose(
                ptr[:, k], h2[:, k * 128 : (k + 1) * 128], self.ident8
            )
        h_cur = self.hring.tile(
            [128, KC, BL], F32R, tag=f"hT{li}", name=f"hT{li}_c"
        )
        nc.vector.tensor_copy(out=h_cur[:], in_=ptr[:])
        nc.sync.dma_start(
            self.hist[:, :, t + 1, :].rearrange("k p b -> p k b"), h_cur[:]
        )
        self.h_prev = h_cur


def build_kernel(T=T_FULL, split=True, interleave=False):
    TOK = T * BL
    NCH = T // CH
    nc = bass.Bass("TRN2")
    x_l = nc.dram_tensor("x", [BL, T, I], F32, kind="ExternalInput")
    # host-pre-transposed weights: [128, KC_in, out_dim] with
    # w[p, k, j] = W_orig[j, k*128 + p] (times 0.5 where prescaled)
    wih0T_d = nc.dram_tensor("wih0T", [128, 1, G], F32R, kind="ExternalInput")
    whh0T_d = nc.dram_tensor("whh0T", [128, KC, G], F32R, kind="ExternalInput")
    wih1T_d = nc.dram_tensor("wih1T", [128, KC, G], F32R, kind="ExternalInput")
    whh1T_d = nc.dram_tensor("whh1T", [128, KC, G], F32R, kind="ExternalInput")
    wqT_d = nc.dram_tensor("wqT", [128, KC, H], F32R, kind="ExternalInput")
    wkT_d = nc.dram_tensor("wkT", [128, KC, H], F32R, kind="ExternalInput")
    wvT_d = nc.dram_tensor("wvT", [128, KC, H], F32R, kind="ExternalInput")
    wfcT_d = nc.dram_tensor("wfcT", [128, KC, O], F32R, kind="ExternalInput")
    b0s = nc.dram_tensor("b0s", [G], F32, kind="ExternalInput")
    b1s = nc.dram_tensor("b1s", [G], F32, kind="ExternalInput")
    bq = nc.dram_tensor("bq", [H], F32, kind="ExternalInput")
    bk = nc.dram_tensor("bk", [H], F32, kind="ExternalInput")
    bv = nc.dram_tensor("bv", [H], F32, kind="ExternalInput")
    bfc = nc.dram_tensor("bfc", [O], F32, kind="ExternalInput")
    y = nc.dram_tensor("y", [BL, O], F32, kind="ExternalOutput")

    with tile.TileContext(nc) as tc:
        with (
            tc.tile_pool(name="big", bufs=1) as big,
            tc.tile_pool(name="state", bufs=1) as state,
            tc.tile_pool(name="hring", bufs=2) as hring,
            tc.tile_pool(name="rot", bufs=3) as rot,
            tc.tile_pool(name="xp", bufs=2) as xp_pool,
            tc.tile_pool(name="hrd", bufs=2) as hrd_pool,
            tc.tile_pool(name="psg", bufs=1, space="PSUM") as psum_gates,
            tc.tile_pool(name="pss", bufs=2, space="PSUM") as psum_small,
            tc.tile_pool(name="dram", bufs=1, space="DRAM") as dram,
        ):
            # ---- identities ----
            ident128 = state.tile([128, 128], F32, tag="id128")
            make_identity(nc, ident128[:])
            ident8 = state.tile([BL, BL], F32, tag="id8")
            make_identity(nc, ident8[:])
            sel128 = state.tile([128, 128], F32R, tag="sel128")
            nc.vector.tensor_copy(out=sel128[:], in_=ident128[:])
            ones = state.tile([1, 128], F32, tag="ones")
            nc.gpsimd.memset(ones[:], 1.0)
            brow = state.tile([1, G], F32, tag="brow")

            # ---- weights (direct DMA of host-transposed forms) ----
            whh0T = big.tile([128, KC, G], F32R, tag="wA")
            nc.sync.dma_start(whh0T[:], whh0T_d[:])
            whh1T = big.tile([128, KC, G], F32R, tag="wA2")
            nc.sync.dma_start(whh1T[:], whh1T_d[:])
            wih1T = big.tile([128, KC, G], F32R, tag="wB")
            nc.sync.dma_start(wih1T[:], wih1T_d[:])
            wih0T = big.tile([128, 1, G], F32R, tag="wB0")
            nc.sync.dma_start(wih0T[:], wih0T_d[:])
            bias_bc = big.tile([128, G], F32, tag="bias")
            nc.sync.dma_start(brow[:], b0s[None, :])
            for n0 in range(0, G, 512):
                ps = psum_small.tile([128, 512], F32, tag="ps_a", name="bcps")
                nc.tensor.matmul(
                    ps[:], ones[:], brow[:, n0 : n0 + 512], start=True, stop=True
                )
                nc.vector.tensor_copy(out=bias_bc[:, n0 : n0 + 512], in_=ps[:])

            # ---- phase 1: xT + xp0 ----
            xt_cols = max(TOK, 4096)
            xT_slot = big.tile([128, xt_cols], F32R, tag="xattn")
            xT = xT_slot[:, :TOK]
            xTv = xT.rearrange("p (t b) -> p t b", b=BL)
            for b in range(BL):
                for tch in range(T // 128):
                    xb = rot.tile([128, I], F32, tag="xb", name="xb")
                    nc.sync.dma_start(xb[:], x_l[b, tch * 128 : (tch + 1) * 128, :])
                    ps = psum_small.tile([128, 128], F32, tag="ps_a", name="xps")
                    nc.tensor.transpose(ps[:], xb[:], ident128)
                    nc.vector.tensor_copy(
                        out=xTv[:, tch * 128 : (tch + 1) * 128, b], in_=ps[:, :128]
                    )
            xp_dram = dram.tile([TOK, G], F32R, tag="xp_dram")
            for mt in range(TOK // 128):
                lhsT = xT[:, mt * 128 : (mt + 1) * 128]
                for n in range(NC_G):
                    ps = psum_small.tile([128, 512], F32, tag="ps_a", name="x0ps")
                    nc.tensor.matmul(
                        ps[:], lhsT, wih0T[:, 0, n * 512 : (n + 1) * 512],
                        start=True, stop=True,
                    )
                    sb = rot.tile([128, 512], F32R, tag="xpout", name="x0sb")
                    nc.vector.tensor_add(
                        out=sb[:], in0=ps[:], in1=bias_bc[:, n * 512 : (n + 1) * 512]
                    )
                    nc.sync.dma_start(
                        xp_dram[mt * 128 : (mt + 1) * 128, n * 512 : (n + 1) * 512],
                        sb[:],
                    )
            # bias1 into the same slot (xp0 fully produced above)
            bias1_bc = big.tile([128, G], F32, tag="bias")
            nc.sync.dma_start(brow[:], b1s[None, :])
            for n0 in range(0, G, 512):
                ps = psum_small.tile([128, 512], F32, tag="ps_a", name="bc1ps")
                nc.tensor.matmul(
                    ps[:], ones[:], brow[:, n0 : n0 + 512], start=True, stop=True
                )
                nc.vector.tensor_copy(out=bias1_bc[:, n0 : n0 + 512], in_=ps[:])

            # ---- interleaved L0 / xp1 / L1 ----
            hist0 = dram.tile([KC, 128, T + 2, BL], F32R, tag="hist0")
            hist1 = dram.tile([KC, 128, T + 2, BL], F32R, tag="hist1")
            xp1_dram = dram.tile([TOK, G], F32R, tag="xp_dram2")
            gates_tile = None
            L0 = LstmState(nc, 0, state, hring, psum_gates, psum_small,
                           whh0T, hist0, sel128, gates_tile)
            L1 = LstmState(nc, 1, state, hring, psum_gates, psum_small,
                           whh1T, hist1, sel128, gates_tile)
            if not interleave:
                for t in range(T):
                    L0.step(t, xp_dram, xp_pool)
            for j in range(NCH + 1):
                if interleave and j >= 1:
                    for t in range((j - 1) * CH, j * CH):
                        L1.step(t, xp1_dram, xp_pool)
                if interleave and j < NCH:
                    for t in range(j * CH, (j + 1) * CH):
                        L0.step(t, xp_dram, xp_pool)
                    # xp1 for chunk j (tokens j*128 .. (j+1)*128)
                    mt = j
                    hch = hrd_pool.tile([128, KC, 128], F32R, tag="hrd", name="hch")
                    nc.sync.dma_start(
                        hch[:],
                        hist0[:, :, 1 + mt * CH : 1 + (mt + 1) * CH, :].rearrange(
                            "k p s b -> p k (s b)"
                        ),
                    )
                    for n in range(NC_G):
                        ps = psum_small.tile([128, 512], F32, tag="ps_a", name="x1ps")
                        for k in range(KC):
                            nc.tensor.matmul(
                                ps[:], hch[:, k, :],
                                wih1T[:, k, n * 512 : (n + 1) * 512],
                                start=(k == 0), stop=(k == KC - 1),
                            )
                        sb = rot.tile([128, 512], F32R, tag="xpout", name="x1sb")
                        nc.vector.tensor_add(
                            out=sb[:], in0=ps[:],
                            in1=bias1_bc[:, n * 512 : (n + 1) * 512],
                        )
                        nc.sync.dma_start(
                            xp1_dram[mt * 128 : (mt + 1) * 128,
                                     n * 512 : (n + 1) * 512],
                            sb[:],
                        )
            if not interleave:
                for mt in range(NCH):
                    hch = hrd_pool.tile([128, KC, 128], F32R, tag="hrd", name="hch")
                    nc.sync.dma_start(
                        hch[:],
                        hist0[:, :, 1 + mt * CH : 1 + (mt + 1) * CH, :].rearrange(
                            "k p s b -> p k (s b)"
                        ),
                    )
                    for n in range(NC_G):
                        ps = psum_small.tile([128, 512], F32, tag="ps_a", name="x1ps")
                        for k in range(KC):
                            nc.tensor.matmul(
                                ps[:], hch[:, k, :],
                                wih1T[:, k, n * 512 : (n + 1) * 512],
                                start=(k == 0), stop=(k == KC - 1),
                            )
                        sb = rot.tile([128, 512], F32R, tag="xpout", name="x1sb")
                        nc.vector.tensor_add(
                            out=sb[:], in0=ps[:],
                            in1=bias1_bc[:, n * 512 : (n + 1) * 512],
                        )
                        nc.sync.dma_start(
                            xp1_dram[mt * 128 : (mt + 1) * 128,
                                     n * 512 : (n + 1) * 512],
                            sb[:],
                        )
                for t in range(T):
                    L1.step(t, xp1_dram, xp_pool)

            # ---- attention + fc ----
            wq_sb = big.tile([128, KC, H], F32R, tag="wA")
            nc.sync.dma_start(wq_sb[:], wqT_d[:])
            wk_sb = big.tile([128, KC, H], F32R, tag="wA2")
            nc.sync.dma_start(wk_sb[:], wkT_d[:])
            wv_sb = big.tile([128, KC, H], F32R, tag="wB")
            nc.sync.dma_start(wv_sb[:], wvT_d[:])
            wfc_sb = big.tile([128, KC, O], F32R, tag="wB0")
            nc.sync.dma_start(wfc_sb[:], wfcT_d[:])
            scale = float(1.0 / np.sqrt(H))
            bq_s = state.tile([128, KC], F32, tag="bq")
            nc.sync.dma_start(bq_s[:], bq.rearrange("(c p) -> p c", p=128))
            nc.vector.tensor_scalar_mul(bq_s[:], bq_s[:], scale)
            bk_s = state.tile([128, KC], F32, tag="bk")
            nc.sync.dma_start(bk_s[:], bk.rearrange("(c p) -> p c", p=128))
            bv_bc = big.tile([128, G], F32, tag="bias")  # [:, :H] used
            nc.sync.dma_start(brow[:, :H], bv[None, :])
            for n0 in range(0, H, 512):
                ps = psum_small.tile([128, 512], F32, tag="ps_a", name="bvps")
                nc.tensor.matmul(
                    ps[:], ones[:], brow[:, n0 : n0 + 512], start=True, stop=True
                )
                nc.vector.tensor_copy(out=bv_bc[:, n0 : n0 + 512], in_=ps[:])
            bfc_s = state.tile([128, 1], F32, tag="bfc")
            nc.sync.dma_start(bfc_s[:], bfc[:, None])

            hq = hring.tile([128, KC, BL], F32R, tag="hT1", name="hq")
            nc.sync.dma_start(
                hq[:], hist1[:, :, T, :].rearrange("k p b -> p k b")
            )
            qT = state.tile([128, KC, BL], F32R, tag="qT")
            for hoc in range(KC):
                ps = psum_small.tile([128, BL], F32, tag="ptr", name="qps")
                for k in range(KC):
                    nc.tensor.matmul(
                        ps[:], wq_sb[:, k, hoc * 128 : (hoc + 1) * 128],
                        hq[:, k, :], start=(k == 0), stop=(k == KC - 1),
                    )
                nc.scalar.activation(
                    qT[:, hoc], ps[:], AF.Identity,
                    bias=bq_s[:, hoc : hoc + 1], scale=scale,
                )
            scores = big.tile([BL, TOK], F32, tag="xattn", name="scores")
            for tk in range(TOK // 512):
                hch = xp_pool.tile([128, KC, 512], F32R, tag="xp", name="hch5")
                nc.sync.dma_start(
                    hch[:],
                    hist1[:, :, 1 + tk * 64 : 1 + (tk + 1) * 64, :].rearrange(
                        "k p s b -> p k (s b)"
                    ),
                )
                ktc = xp_pool.tile([128, KC, 512], F32R, tag="xp", name="ktc")
                for hoc in range(KC):
                    ps = psum_small.tile([128, 512], F32, tag="ps_a", name="kps")
                    for k in range(KC):
                        nc.tensor.matmul(
                            ps[:], wk_sb[:, k, hoc * 128 : (hoc + 1) * 128],
                            hch[:, k, :], start=(k == 0), stop=(k == KC - 1),
                        )
                    nc.scalar.activation(
                        ktc[:, hoc], ps[:], AF.Identity,
                        bias=bk_s[:, hoc : hoc + 1],
                    )
                scp = psum_small.tile([BL, 512], F32, tag="ps_a", name="scp")
                for k in range(KC):
                    nc.tensor.matmul(
                        scp[:], qT[:, k, :], ktc[:, k, :],
                        start=(k == 0), stop=(k == KC - 1),
                    )
                nc.vector.tensor_copy(
                    out=scores[:, tk * 512 : (tk + 1) * 512], in_=scp[:]
                )
            # masked softmax on the b-diagonal (mask = ident8 broadcast over t)
            BIGC = 30000.0
            attn = scores
            av = attn.rearrange("p (t b) -> p t b", b=BL)
            nc.vector.scalar_tensor_tensor(
                out=av[:], in0=av[:], scalar=BIGC,
                in1=ident8[:, None, :].to_broadcast((BL, T, BL)),
                op0=ALU.add, op1=ALU.mult,
            )
            negmax = state.tile([BL, 1], F32, tag="negmax")
            nc.vector.tensor_reduce(
                out=negmax[:], in_=attn[:],
                axis=mybir.AxisListType.X, op=ALU.max, negate=True,
            )
            sumexp = state.tile([BL, 1], F32, tag="sumexp")
            nc.scalar.activation(
                attn[:], attn[:], AF.Exp, bias=negmax[:], accum_out=sumexp[:]
            )
            rec = state.tile([BL, 1], F32, tag="rec")
            nc.vector.reciprocal(out=rec[:], in_=sumexp[:])
            nc.vector.tensor_scalar_mul(attn[:], attn[:], rec[:])
            attnT = state.tile([128, TOK // 128, BL], F32R, tag="attnT")
            for tk2 in range(TOK // 128):
                ps = psum_small.tile([128, BL], F32, tag="ptr", name="aps")
                nc.tensor.transpose(
                    ps[:], attn[:, tk2 * 128 : (tk2 + 1) * 128], ident8
                )
                nc.vector.tensor_copy(out=attnT[:, tk2, :], in_=ps[:])
            ctx_ps = [
                psum_gates.tile([128, 512], F32, tag=f"gate{hoc}", name=f"cps{hoc}")[
                    :, 0:BL
                ]
                for hoc in range(KC)
            ]
            for tk2 in range(TOK // 128):
                hch = hrd_pool.tile([128, KC, 128], F32R, tag="hrd", name="hch5b")
                nc.sync.dma_start(
                    hch[:],
                    hist1[:, :, 1 + tk2 * CH : 1 + (tk2 + 1) * CH, :].rearrange(
                        "k p s b -> p k (s b)"
                    ),
                )
                vps = psum_small.tile([128, 512], F32, tag="ps_a", name="vps")
                for k in range(KC):
                    nc.tensor.matmul(
                        vps[:], hch[:, k, :], wv_sb[:, k, :],
                        start=(k == 0), stop=(k == KC - 1),
                    )
                vc = rot.tile([128, 512], F32R, tag="xpout", name="vc")
                nc.vector.tensor_add(out=vc[:], in0=vps[:], in1=bv_bc[:, :H])
                for hoc in range(KC):
                    nc.tensor.matmul(
                        ctx_ps[hoc], vc[:, hoc * 128 : (hoc + 1) * 128],
                        attnT[:, tk2, :],
                        start=(tk2 == 0), stop=(tk2 == TOK // 128 - 1),
                    )
            ctxT = state.tile([128, KC, BL], F32R, tag="ctxT")
            for hoc in range(KC):
                nc.scalar.activation(ctxT[:, hoc], ctx_ps[hoc], AF.Copy)
            ops = psum_small.tile([128, BL], F32, tag="ptr", name="ops")
            for k in range(KC):
                nc.tensor.matmul(
                    ops[:], wfc_sb[:, k, :], ctxT[:, k, :],
                    start=(k == 0), stop=(k == KC - 1),
                )
            outT_sb = state.tile([128, BL], F32, tag="outTsb")
            nc.scalar.activation(outT_sb[:], ops[:], AF.Identity, bias=bfc_s[:])
            fps = psum_small.tile([BL, O], F32, tag="ps_a", name="fps")
            nc.tensor.transpose(fps[:], outT_sb[:], ident128)
            y_sb = state.tile([BL, O], F32, tag="ysb")
            nc.vector.tensor_copy(out=y_sb[:], in_=fps[:])
            nc.sync.dma_start(y[:], y_sb[:])

    if split:
        split_wide_waits(nc)
    return nc


_NC_CACHE = {}


def _get_nc(T=T_FULL):
    if T not in _NC_CACHE:
        _NC_CACHE[T] = build_kernel(T)
    return _NC_CACHE[T]


def _prep_t(w, scale=1.0):
    """[out_dim, in_dim] -> [128, in_dim//128, out_dim] host pre-transpose."""
    out_dim, in_dim = w.shape
    wt = (w.T * scale).astype(np.float32)  # [in, out]
    return np.ascontiguousarray(
        wt.reshape(in_dim // 128, 128, out_dim).transpose(1, 0, 2)
    )


def prepare_host_inputs(inputs):
    m = {}
    m["wih0T"] = _prep_t(np.asarray(inputs["W_ih0"])).reshape(128, 1, G)
    m["whh0T"] = _prep_t(np.asarray(inputs["W_hh0"]), 0.5)
    m["wih1T"] = _prep_t(np.asarray(inputs["W_ih1"]), 0.5)
    m["whh1T"] = _prep_t(np.asarray(inputs["W_hh1"]), 0.5)
    m["wqT"] = _prep_t(np.asarray(inputs["Wq"]), 0.5)
    m["wkT"] = _prep_t(np.asarray(inputs["Wk"]), 0.5)
    m["wvT"] = _prep_t(np.asarray(inputs["Wv"]), 0.5)
    m["wfcT"] = _prep_t(np.asarray(inputs["Wfc"]))
    m["b0s"] = (np.asarray(inputs["b_ih0"]) + np.asarray(inputs["b_hh0"])).astype(
        np.float32
    )
    m["b1s"] = (np.asarray(inputs["b_ih1"]) + np.asarray(inputs["b_hh1"])).astype(
        np.float32
    )
    for n in ["bq", "bk", "bv", "bfc"]:
        m[n] = np.ascontiguousarray(np.asarray(inputs[n], dtype=np.float32))
    return m


def build_in_maps(inputs):
    x = np.asarray(inputs["x"], dtype=np.float32)
    shared = prepare_host_inputs(inputs)
    in_maps = []
    for c in range(N_CORES):
        m = dict(shared)
        m["x"] = np.ascontiguousarray(x[c * BL : (c + 1) * BL])
        in_maps.append(m)
    return in_maps


def kernel(**inputs):
    x = np.asarray(inputs["x"])
    nc = _get_nc(x.shape[1])
    in_maps = build_in_maps(inputs)
    res = run_bass_kernel_spmd(nc, in_maps, core_ids=list(range(N_CORES)))
    return np.concatenate([res.results[c]["y"] for c in range(N_CORES)], axis=0)


# revision 25
# speedup vs baseline: 1.6435x; 1.0043x over previous
"""Trainium2 Bass kernel for nn_AttenLSTMModel (2-layer LSTM + last-query
attention + FC head).

kernel(**inputs): FULL numpy inputs (B=64) -> FULL [64, 128] output.
Batch sharded 8-way across the 8 NeuronCores (pure SPMD, one NEFF).

v1.5 design notes (per core, local batch BL=8, token index = t*BL + b):
 - Weights arrive HOST-pre-transposed (and 0.5-prescaled where needed).
 - All gate activations are Tanh-only (sigmoid(z) = 0.5*tanh(z/2)+0.5) so
   the ACT engine never reloads its LUT table. Affine factors folded:
   track C2 = 2c and h2 = 2h; pre-scale Whh^T, Wih1^T, Wq/Wk/Wv^T by 0.5.
     A    = (th_i + 1) * th_g          (= 2 i' g')
     B    = (th_f + 1) * C2_old        (= 4 f' c_old)
     C2   = 0.5*B + A
     th_C = tanh(0.5 * C2)             (= tanh(c))
     h2   = (th_o + 1) * th_C          (= 2 h)
 - xp preloaded into PSUM via identity-selector matmul; fp32r matmuls.
 - LSTM0 / xp1-production / LSTM1 interleaved in 16-step chunks so one
   layer's matmul stream fills the other layer's elementwise-tail bubbles.
 - h_t^T history spilled to DRAM (hist0/hist1), read back in chunks.
"""

import os

os.environ.setdefault("JAX_PLATFORMS", "axon,cpu")

import numpy as np

import concourse.bass as bass
import concourse.mybir as mybir
import concourse.tile as tile
from concourse.bass_utils import run_bass_kernel_spmd
from concourse.masks import make_identity

F32 = mybir.dt.float32
F32R = mybir.dt.float32r
AF = mybir.ActivationFunctionType
ALU = mybir.AluOpType

B, T_FULL, I, H, O = 64, 512, 128, 512, 128
G = 4 * H
N_CORES = 8
BL = B // N_CORES  # 8
KC = H // 128  # 4
NC_G = G // 512  # 4
HB = KC * BL  # 32
CH = 16  # interleave chunk: steps per chunk (= 128 tokens)


def split_wide_waits(nc):
    """This container's walrus allows only 1 sem wait on CTRL-class
    instructions (Drain/NoOp) and 2 on compute/DMA instructions. Move
    excess waits onto preceding same-engine wait-only NoOps."""
    ctrl_types = (mybir.InstDrain, mybir.InstNoOp, mybir.InstEventSemaphore)
    for f in nc.m.functions:
        for bb in f.blocks:
            insts = bb.instructions
            changed = False
            out = []
            for ins in insts:
                si = getattr(ins, "sync_info", None)
                limit = 1
                if si and si.on_wait and len(si.on_wait) > limit:
                    waits = list(si.on_wait)
                    extra, keep = waits[:-limit], waits[-limit:]
                    for j, w in enumerate(extra):
                        nd = mybir.InstNoOp(name=f"{ins.name}-wsplit{j}")
                        nd.engine = ins.engine
                        nd.sync_info = mybir.SyncInfo(on_wait=[w], on_update=[])
                        out.append(nd)
                    ins.sync_info = mybir.SyncInfo(
                        on_wait=keep, on_update=list(si.on_update)
                    )
                    changed = True
                out.append(ins)
            if changed:
                bb.instructions = out


class LstmState:
    """Per-layer recurrence state + step emitter."""

    def __init__(self, nc, li, state, hring, psum_gates, psum_small, whhT,
                 hist_dram, sel128, gates_tile):
        self.nc = nc
        self.li = li
        self.pb = 0
        self.state = state
        self.hring = hring
        self.psg = psum_gates
        self.pss = psum_small
        self.whhT = whhT
        self.hist = hist_dram
        self.sel128 = sel128
        id8t = state.tile([40, BL], F32, tag=f"id8_{li}", name=f"id8_{li}")
        make_identity(nc, id8t[self.pb : self.pb + BL, :])
        self.ident8 = id8t[self.pb : self.pb + BL, :]
        c2t = state.tile([40, H], F32, tag=f"c{li}", name=f"c{li}")
        self.c2 = c2t[self.pb : self.pb + BL]
        nc.gpsimd.memset(self.c2[:], 0.0)
        zro = state.tile([128, KC, BL], F32, tag="zro", name="zro")
        nc.gpsimd.memset(zro[:], 0.0)
        self.h_prev = hring.tile(
            [128, KC, BL], F32R, tag=f"hT{li}", name=f"hT{li}_init"
        )
        nc.vector.tensor_copy(out=self.h_prev[:], in_=zro[:])
        self.xp_tile = None

    def _sbt(self, tag):
        t = self.state.tile([40, 512], F32, tag=f"{tag}{self.li}",
                            name=f"{tag}{self.li}")
        return t[self.pb : self.pb + BL]

    def step(self, t, xp_dram, xp_pool):
        nc = self.nc
        li = self.li
        sel = t % CH
        if sel == 0:
            self.xp_tile = xp_pool.tile(
                [128, G], F32R, tag="xp", name=f"xp{li}c"
            )
            nc.sync.dma_start(
                self.xp_tile[:], xp_dram[t * BL : t * BL + 128, :]
            )
        gates = {}
        for n in (0, 2, 1, 3):  # i, f, g, o slices; issue order i,g,f,o
            gp = self.psg.tile([BL, 512], F32, tag=f"gate{n}", name=f"g{n}_{self.li}")
            gates[n] = gp
            nc.tensor.matmul(
                gp[:],
                self.sel128[:, sel * BL : (sel + 1) * BL],
                self.xp_tile[:, n * 512 : (n + 1) * 512],
                start=True,
                stop=False,
            )
            for k in range(KC):
                nc.tensor.matmul(
                    gp[:],
                    self.h_prev[:, k, :],
                    self.whhT[:, k, n * 512 : (n + 1) * 512],
                    start=False,
                    stop=(k == KC - 1),
                )
        th = {}
        # i, f, o: tanh(z/2); g: tanh(z)
        for n, sc in ((0, 0.5), (2, 1.0), (1, 0.5), (3, 0.5)):
            a = self._sbt(f"th{n}_")
            th[n] = a
            nc.scalar.activation(a[:], gates[n][:], AF.Tanh, scale=sc)
        A = self._sbt("A")
        nc.vector.scalar_tensor_tensor(
            out=A[:], in0=th[0][:], scalar=1.0, in1=th[2][:],
            op0=ALU.add, op1=ALU.mult,
        )
        Bt = self._sbt("B")
        nc.vector.scalar_tensor_tensor(
            out=Bt[:], in0=th[1][:], scalar=1.0, in1=self.c2[:],
            op0=ALU.add, op1=ALU.mult,
        )
        nc.vector.scalar_tensor_tensor(
            out=self.c2[:], in0=Bt[:], scalar=0.5, in1=A[:],
            op0=ALU.mult, op1=ALU.add,
        )
        thc = self._sbt("thc")
        nc.scalar.activation(thc[:], self.c2[:], AF.Tanh, scale=0.5)
        h2 = self._sbt("h")
        nc.vector.scalar_tensor_tensor(
            out=h2[:], in0=th[3][:], scalar=1.0, in1=thc[:],
            op0=ALU.add, op1=ALU.mult,
        )
        ptr = self.pss.tile([128, KC, BL], F32, tag="ptr", name=f"ptr{li}")
        for k in range(KC):
            nc.tensor.transpose(
                ptr[:, k], h2[:, k * 128 : (k + 1) * 128], self.ident8
            )
        h_cur = self.hring.tile(
            [128, KC, BL], F32R, tag=f"hT{li}", name=f"hT{li}_c"
        )
        nc.vector.tensor_copy(out=h_cur[:], in_=ptr[:])
        nc.sync.dma_start(
            self.hist[:, :, t + 1, :].rearrange("k p b -> p k b"), h_cur[:]
        )
        self.h_prev = h_cur


def build_kernel(T=T_FULL, split=True, interleave=False):
    TOK = T * BL
    NCH = T // CH
    nc = bass.Bass("TRN2")
    x_l = nc.dram_tensor("x", [BL, T, I], F32, kind="ExternalInput")
    # host-pre-transposed weights: [128, KC_in, out_dim] with
    # w[p, k, j] = W_orig[j, k*128 + p] (times 0.5 where prescaled)
    wih0T_d = nc.dram_tensor("wih0T", [128, 1, G], F32R, kind="ExternalInput")
    whh0T_d = nc.dram_tensor("whh0T", [128, KC, G], F32R, kind="ExternalInput")
    wih1T_d = nc.dram_tensor("wih1T", [128, KC, G], F32R, kind="ExternalInput")
    whh1T_d = nc.dram_tensor("whh1T", [128, KC, G], F32R, kind="ExternalInput")
    wqT_d = nc.dram_tensor("wqT", [128, KC, H], F32R, kind="ExternalInput")
    wkT_d = nc.dram_tensor("wkT", [128, KC, H], F32R, kind="ExternalInput")
    wvT_d = nc.dram_tensor("wvT", [128, KC, H], F32R, kind="ExternalInput")
    wfcT_d = nc.dram_tensor("wfcT", [128, KC, O], F32R, kind="ExternalInput")
    b0s = nc.dram_tensor("b0s", [G], F32, kind="ExternalInput")
    b1s = nc.dram_tensor("b1s", [G], F32, kind="ExternalInput")
    bq = nc.dram_tensor("bq", [H], F32, kind="ExternalInput")
    bk = nc.dram_tensor("bk", [H], F32, kind="ExternalInput")
    bv = nc.dram_tensor("bv", [H], F32, kind="ExternalInput")
    bfc = nc.dram_tensor("bfc", [O], F32, kind="ExternalInput")
    y = nc.dram_tensor("y", [BL, O], F32, kind="ExternalOutput")

    with tile.TileContext(nc) as tc:
        with (
            tc.tile_pool(name="big", bufs=1) as big,
            tc.tile_pool(name="state", bufs=1) as state,
            tc.tile_pool(name="hring", bufs=2) as hring,
            tc.tile_pool(name="rot", bufs=3) as rot,
            tc.tile_pool(name="xp", bufs=2) as xp_pool,
            tc.tile_pool(name="hrd", bufs=2) as hrd_pool,
            tc.tile_pool(name="psg", bufs=1, space="PSUM") as psum_gates,
            tc.tile_pool(name="pss", bufs=2, space="PSUM") as psum_small,
            tc.tile_pool(name="dram", bufs=1, space="DRAM") as dram,
        ):
            # ---- identities ----
            ident128 = state.tile([128, 128], F32, tag="id128")
            make_identity(nc, ident128[:])
            ident8 = state.tile([BL, BL], F32, tag="id8")
            make_identity(nc, ident8[:])
            sel128 = state.tile([128, 128], F32R, tag="sel128")
            nc.vector.tensor_copy(out=sel128[:], in_=ident128[:])
            ones = state.tile([1, 128], F32, tag="ones")
            nc.gpsimd.memset(ones[:], 1.0)
            brow = state.tile([1, G], F32, tag="brow")

            # ---- weights (direct DMA of host-transposed forms) ----
            whh0T = big.tile([128, KC, G], F32R, tag="wA")
            nc.sync.dma_start(whh0T[:], whh0T_d[:])
            whh1T = big.tile([128, KC, G], F32R, tag="wA2")
            nc.sync.dma_start(whh1T[:], whh1T_d[:])
            wih1T = big.tile([128, KC, G], F32R, tag="wB")
            nc.sync.dma_start(wih1T[:], wih1T_d[:])
            wih0T = big.tile([128, 1, G], F32R, tag="wB0")
            nc.sync.dma_start(wih0T[:], wih0T_d[:])
            bias_bc = big.tile([128, G], F32, tag="bias")
            nc.sync.dma_start(brow[:], b0s[None, :])
            for n0 in range(0, G, 512):
                ps = psum_small.tile([128, 512], F32, tag="ps_a", name="bcps")
                nc.tensor.matmul(
                    ps[:], ones[:], brow[:, n0 : n0 + 512], start=True, stop=True
                )
                nc.vector.tensor_copy(out=bias_bc[:, n0 : n0 + 512], in_=ps[:])

            # ---- phase 1: xT + xp0 ----
            xt_cols = max(TOK, 4096)
            xT_slot = big.tile([128, xt_cols], F32R, tag="xattn")
            xT = xT_slot[:, :TOK]
            xTv = xT.rearrange("p (t b) -> p t b", b=BL)
            for b in range(BL):
                for tch in range(T // 128):
                    xb = rot.tile([128, I], F32, tag="xb", name="xb")
                    nc.sync.dma_start(xb[:], x_l[b, tch * 128 : (tch + 1) * 128, :])
                    ps = psum_small.tile([128, 128], F32, tag="ps_a", name="xps")
                    nc.tensor.transpose(ps[:], xb[:], ident128)
                    nc.vector.tensor_copy(
                        out=xTv[:, tch * 128 : (tch + 1) * 128, b], in_=ps[:, :128]
                    )
            xp_dram = dram.tile([TOK, G], F32R, tag="xp_dram")
            for mt in range(TOK // 128):
                lhsT = xT[:, mt * 128 : (mt + 1) * 128]
                for n in range(NC_G):
                    ps = psum_small.tile([128, 512], F32, tag="ps_a", name="x0ps")
                    nc.tensor.matmul(
                        ps[:], lhsT, wih0T[:, 0, n * 512 : (n + 1) * 512],
                        start=True, stop=True,
                    )
                    sb = rot.tile([128, 512], F32R, tag="xpout", name="x0sb")
                    nc.vector.tensor_add(
                        out=sb[:], in0=ps[:], in1=bias_bc[:, n * 512 : (n + 1) * 512]
                    )
                    nc.sync.dma_start(
                        xp_dram[mt * 128 : (mt + 1) * 128, n * 512 : (n + 1) * 512],
                        sb[:],
                    )
            # bias1 into the same slot (xp0 fully produced above)
            bias1_bc = big.tile([128, G], F32, tag="bias")
            nc.sync.dma_start(brow[:], b1s[None, :])
            for n0 in range(0, G, 512):
                ps = psum_small.tile([128, 512], F32, tag="ps_a", name="bc1ps")
                nc.tensor.matmul(
                    ps[:], ones[:], brow[:, n0 : n0 + 512], start=True, stop=True
                )
                nc.vector.tensor_copy(out=bias1_bc[:, n0 : n0 + 512], in_=ps[:])

            # ---- interleaved L0 / xp1 / L1 ----
            hist0 = dram.tile([KC, 128, T + 2, BL], F32R, tag="hist0")
            hist1 = dram.tile([KC, 128, T + 2, BL], F32R, tag="hist1")
            xp1_dram = dram.tile([TOK, G], F32R, tag="xp_dram2")
            gates_tile = None
            L0 = LstmState(nc, 0, state, hring, psum_gates, psum_small,
                           whh0T, hist0, sel128, gates_tile)
            L1 = LstmState(nc, 1, state, hring, psum_gates, psum_small,
                           whh1T, hist1, sel128, gates_tile)
            if not interleave:
                for t in range(T):
                    L0.step(t, xp_dram, xp_pool)
            for j in range(NCH + 1):
                if interleave and j >= 1:
                    for t in range((j - 1) * CH, j * CH):
                        L1.step(t, xp1_dram, xp_pool)
                if interleave and j < NCH:
                    for t in range(j * CH, (j + 1) * CH):
                        L0.step(t, xp_dram, xp_pool)
                    # xp1 for chunk j (tokens j*128 .. (j+1)*128)
                    mt = j
                    hch = hrd_pool.tile([128, KC, 128], F32R, tag="hrd", name="hch")
                    nc.sync.dma_start(
                        hch[:],
                        hist0[:, :, 1 + mt * CH : 1 + (mt + 1) * CH, :].rearrange(
                            "k p s b -> p k (s b)"
                        ),
                    )
                    for n in range(NC_G):
                        ps = psum_small.tile([128, 512], F32, tag="ps_a", name="x1ps")
                        for k in range(KC):
                            nc.tensor.matmul(
                                ps[:], hch[:, k, :],
                                wih1T[:, k, n * 512 : (n + 1) * 512],
                                start=(k == 0), stop=(k == KC - 1),
                            )
                        sb = rot.tile([128, 512], F32R, tag="xpout", name="x1sb")
                        nc.vector.tensor_add(
                            out=sb[:], in0=ps[:],
                            in1=bias1_bc[:, n * 512 : (n + 1) * 512],
                        )
                        nc.sync.dma_start(
                            xp1_dram[mt * 128 : (mt + 1) * 128,
                                     n * 512 : (n + 1) * 512],
                            sb[:],
                        )
            if not interleave:
                for mt in range(NCH):
                    hch = hrd_pool.tile([128, KC, 128], F32R, tag="hrd", name="hch")
                    nc.sync.dma_start(
                        hch[:],
                        hist0[:, :, 1 + mt * CH : 1 + (mt + 1) * CH, :].rearrange(
                            "k p s b -> p k (s b)"
                        ),
                    )
                    for n in range(NC_G):
                        ps = psum_small.tile([128, 512], F32, tag="ps_a", name="x1ps")
                        for k in range(KC):
                            nc.tensor.matmul(
                                ps[:], hch[:, k, :],
                                wih1T[:, k, n * 512 : (n + 1) * 512],
                                start=(k == 0), stop=(k == KC - 1),
                            )
                        sb = rot.tile([128, 512], F32R, tag="xpout", name="x1sb")
                        nc.vector.tensor_add(
                            out=sb[:], in0=ps[:],
                            in1=bias1_bc[:, n * 512 : (n + 1) * 512],
                        )
                        nc.sync.dma_start(
                            xp1_dram[mt * 128 : (mt + 1) * 128,
                                     n * 512 : (n + 1) * 512],
                            sb[:],
                        )
                for t in range(T):
                    L1.step(t, xp1_dram, xp_pool)

            # ---- attention + fc ----
            wq_sb = big.tile([128, KC, H], F32R, tag="wA")
            nc.sync.dma_start(wq_sb[:], wqT_d[:])
            wk_sb = big.tile([128, KC, H], F32R, tag="wA2")
            nc.sync.dma_start(wk_sb[:], wkT_d[:])
            wv_sb = big.tile([128, KC, H], F32R, tag="wB")
            nc.sync.dma_start(wv_sb[:], wvT_d[:])
            wfc_sb = big.tile([128, KC, O], F32R, tag="wB0")
            nc.sync.dma_start(wfc_sb[:], wfcT_d[:])
            scale = float(1.0 / np.sqrt(H))
            bq_s = state.tile([128, KC], F32, tag="bq")
            nc.sync.dma_start(bq_s[:], bq.rearrange("(c p) -> p c", p=128))
            nc.vector.tensor_scalar_mul(bq_s[:], bq_s[:], scale)
            bk_s = state.tile([128, KC], F32, tag="bk")
            nc.sync.dma_start(bk_s[:], bk.rearrange("(c p) -> p c", p=128))
            bv_bc = big.tile([128, G], F32, tag="bias")  # [:, :H] used
            nc.sync.dma_start(brow[:, :H], bv[None, :])
            for n0 in range(0, H, 512):
                ps = psum_small.tile([128, 512], F32, tag="ps_a", name="bvps")
                nc.tensor.matmul(
                    ps[:], ones[:], brow[:, n0 : n0 + 512], start=True, stop=True
                )
                nc.vector.tensor_copy(out=bv_bc[:, n0 : n0 + 512], in_=ps[:])
            bfc_s = state.tile([128, 1], F32, tag="bfc")
            nc.sync.dma_start(bfc_s[:], bfc[:, None])

            hq = hring.tile([128, KC, BL], F32R, tag="hT1", name="hq")
            nc.sync.dma_start(
                hq[:], hist1[:, :, T, :].rearrange("k p b -> p k b")
            )
            qT = state.tile([128, KC, BL], F32R, tag="qT")
            for hoc in range(KC):
                ps = psum_small.tile([128, BL], F32, tag="ptr", name="qps")
                for k in range(KC):
                    nc.tensor.matmul(
                        ps[:], wq_sb[:, k, hoc * 128 : (hoc + 1) * 128],
                        hq[:, k, :], start=(k == 0), stop=(k == KC - 1),
                    )
                nc.scalar.activation(
                    qT[:, hoc], ps[:], AF.Identity,
                    bias=bq_s[:, hoc : hoc + 1], scale=scale,
                )
            scores = big.tile([BL, TOK], F32, tag="xattn", name="scores")
            for tk in range(TOK // 512):
                hch = xp_pool.tile([128, KC, 512], F32R, tag="xp", name="hch5")
                nc.sync.dma_start(
                    hch[:],
                    hist1[:, :, 1 + tk * 64 : 1 + (tk + 1) * 64, :].rearrange(
                        "k p s b -> p k (s b)"
                    ),
                )
                ktc = xp_pool.tile([128, KC, 512], F32R, tag="xp", name="ktc")
                for hoc in range(KC):
                    ps = psum_small.tile([128, 512], F32, tag="ps_a", name="kps")
                    for k in range(KC):
                        nc.tensor.matmul(
                            ps[:], wk_sb[:, k, hoc * 128 : (hoc + 1) * 128],
                            hch[:, k, :], start=(k == 0), stop=(k == KC - 1),
                        )
                    nc.scalar.activation(
                        ktc[:, hoc], ps[:], AF.Identity,
                        bias=bk_s[:, hoc : hoc + 1],
                    )
                scp = psum_small.tile([BL, 512], F32, tag="ps_a", name="scp")
                for k in range(KC):
                    nc.tensor.matmul(
                        scp[:], qT[:, k, :], ktc[:, k, :],
                        start=(k == 0), stop=(k == KC - 1),
                    )
                nc.vector.tensor_copy(
                    out=scores[:, tk * 512 : (tk + 1) * 512], in_=scp[:]
                )
            # masked softmax on the b-diagonal (mask = ident8 broadcast over t)
            BIGC = 30000.0
            attn = scores
            av = attn.rearrange("p (t b) -> p t b", b=BL)
            nc.vector.scalar_tensor_tensor(
                out=av[:], in0=av[:], scalar=BIGC,
                in1=ident8[:, None, :].to_broadcast((BL, T, BL)),
                op0=ALU.add, op1=ALU.mult,
            )
            negmax = state.tile([BL, 1], F32, tag="negmax")
            nc.vector.tensor_reduce(
                out=negmax[:], in_=attn[:],
                axis=mybir.AxisListType.X, op=ALU.max, negate=True,
            )
            sumexp = state.tile([BL, 1], F32, tag="sumexp")
            nc.scalar.activation(
                attn[:], attn[:], AF.Exp, bias=negmax[:], accum_out=sumexp[:]
            )
            rec = state.tile([BL, 1], F32, tag="rec")
            nc.vector.reciprocal(out=rec[:], in_=sumexp[:])
            nc.vector.tensor_scalar_mul(attn[:], attn[:], rec[:])
            attnT = state.tile([128, TOK // 128, BL], F32R, tag="attnT")
            for tk2 in range(TOK // 128):
                ps = psum_small.tile([128, BL], F32, tag="ptr", name="aps")
                nc.tensor.transpose(
                    ps[:], attn[:, tk2 * 128 : (tk2 + 1) * 128], ident8
                )
                nc.vector.tensor_copy(out=attnT[:, tk2, :], in_=ps[:])
            ctx_ps = [
                psum_gates.tile([128, 512], F32, tag=f"gate{hoc}", name=f"cps{hoc}")[
                    :, 0:BL
                ]
                for hoc in range(KC)
            ]
            for tk2 in range(TOK // 128):
                hch = hrd_pool.tile([128, KC, 128], F32R, tag="hrd", name="hch5b")
                nc.sync.dma_start(
                    hch[:],
                    hist1[:, :, 1 + tk2 * CH : 1 + (tk2 + 1) * CH, :].rearrange(
                        "k p s b -> p k (s b)"
                    ),
                )
                vps = psum_small.tile([128, 512], F32, tag="ps_a", name="vps")
                for k in range(KC):
                    nc.tensor.matmul(
                        vps[:], hch[:, k, :], wv_sb[:, k, :],
                        start=(k == 0), stop=(k == KC - 1),
                    )
                vc = rot.tile([128, 512], F32R, tag="xpout", name="vc")
                nc.vector.tensor_add(out=vc[:], in0=vps[:], in1=bv_bc[:, :H])
                for hoc in range(KC):
                    nc.tensor.matmul(
                        ctx_ps[hoc], vc[:, hoc * 128 : (hoc + 1) * 128],
                        attnT[:, tk2, :],
                        start=(tk2 == 0), stop=(tk2 == TOK // 128 - 1),
                    )
            ctxT = state.tile([128, KC, BL], F32R, tag="ctxT")
            for hoc in range(KC):
                nc.scalar.activation(ctxT[:, hoc], ctx_ps[hoc], AF.Copy)
            ops = psum_small.tile([128, BL], F32, tag="ptr", name="ops")
            for k in range(KC):
                nc.tensor.matmul(
                    ops[:], wfc_sb[:, k, :], ctxT[:, k, :],
                    start=(k == 0), stop=(k == KC - 1),
                )
            outT_sb = state.tile([128, BL], F32, tag="outTsb")
            nc.scalar.activation(outT_sb[:], ops[:], AF.Identity, bias=bfc_s[:])
            fps = psum_small.tile([BL, O], F32, tag="ps_a", name="fps")
            nc.tensor.transpose(fps[:], outT_sb[:], ident128)
            y_sb = state.tile([BL, O], F32, tag="ysb")
            nc.vector.tensor_copy(out=y_sb[:], in_=fps[:])
            nc.sync.dma_start(y[:], y_sb[:])

    if split:
        split_wide_waits(nc)
    return nc


_NC_CACHE = {}


def _get_nc(T=T_FULL):
    if T not in _NC_CACHE:
        _NC_CACHE[T] = build_kernel(T)
    return _NC_CACHE[T]


def _prep_t(w, scale=1.0):
    """[out_dim, in_dim] -> [128, in_dim//128, out_dim] host pre-transpose."""
    out_dim, in_dim = w.shape
    wt = (w.T * scale).astype(np.float32)  # [in, out]
    return np.ascontiguousarray(
        wt.reshape(in_dim // 128, 128, out_dim).transpose(1, 0, 2)
    )


def prepare_host_inputs(inputs):
    m = {}
    m["wih0T"] = _prep_t(np.asarray(inputs["W_ih0"])).reshape(128, 1, G)
    m["whh0T"] = _prep_t(np.asarray(inputs["W_hh0"]), 0.5)
    m["wih1T"] = _prep_t(np.asarray(inputs["W_ih1"]), 0.5)
    m["whh1T"] = _prep_t(np.asarray(inputs["W_hh1"]), 0.5)
    m["wqT"] = _prep_t(np.asarray(inputs["Wq"]), 0.5)
    m["wkT"] = _prep_t(np.asarray(inputs["Wk"]), 0.5)
    m["wvT"] = _prep_t(np.asarray(inputs["Wv"]), 0.5)
    m["wfcT"] = _prep_t(np.asarray(inputs["Wfc"]))
    m["b0s"] = (np.asarray(inputs["b_ih0"]) + np.asarray(inputs["b_hh0"])).astype(
        np.float32
    )
    m["b1s"] = (np.asarray(inputs["b_ih1"]) + np.asarray(inputs["b_hh1"])).astype(
        np.float32
    )
    for n in ["bq", "bk", "bv", "bfc"]:
        m[n] = np.ascontiguousarray(np.asarray(inputs[n], dtype=np.float32))
    return m


def build_in_maps(inputs):
    x = np.asarray(inputs["x"], dtype=np.float32)
    shared = prepare_host_inputs(inputs)
    in_maps = []
    for c in range(N_CORES):
        m = dict(shared)
        m["x"] = np.ascontiguousarray(x[c * BL : (c + 1) * BL])
        in_maps.append(m)
    return in_maps


def kernel(**inputs):
    x = np.asarray(inputs["x"])
    nc = _get_nc(x.shape[1])
    in_maps = build_in_maps(inputs)
    res = run_bass_kernel_spmd(nc, in_maps, core_ids=list(range(N_CORES)))
    return np.concatenate([res.results[c]["y"] for c in range(N_CORES)], axis=0)
